# revision 1
# baseline (speedup 1.0000x reference)
"""Causal MHSA Trainium2 kernel (8 NeuronCores) — v3.

Sharding: core c = 4*b + g handles batch b and head-group g (4 of 16
heads); host sums the 4 head-group partial projections per batch.

v3 (vs v2):
- Loads batched into ~13 large strided DMAs (x via the Pool/SWDGE queue
  which bypasses the shared HWDGE; weights on the SP queue), ordered so
  the first projection starts ~4us in and attention(1,0) is pulled into
  the startup window before x block 1 lands.
- Softmax normalization broadcasts the reciprocal row across partitions
  with a 1-row PE matmul (ones row x recip row -> PSUM) instead of a
  DRAM DMA round-trip; the broadcast+multiply is deferred into the next
  attention block so PE never waits on the DVE reciprocal.
- P (exp output) and V are bf16: ctx matmuls run 1 cycle/col at any
  width (fp32r pays 4x under 256 cols), diagonal score matmuls are
  widened to >=256 cols, and the DVE mask-multiplies run in 2x mode.
- Fillers are fine-grained units (~2 matmuls each), popped 2 per kt
  iteration, so the ACT exp pacing deficit (~186ns/kt) is absorbed by
  projection/output work instead of PE idling.
- Output projection staging copies all on DVE (ACT only runs exps); the
  final 4 q-tiles DMA straight from PSUM.
"""

import json

import numpy as np

import concourse.bass as bass
import concourse.mybir as mybir
import concourse.tile as tile
from concourse.bass_utils import run_bass_kernel_spmd

# ---------------------------------------------------------------------------
# Workaround: this container's walrus rejects instructions carrying more
# than one semaphore wait ("Too many sync wait commands", e.g. on the
# TileContext final drain). Split every multi-wait instruction into
# single-wait NoOps on the same engine placed immediately before it.
# ---------------------------------------------------------------------------


def _split_multiwait_bir(bir_bytes: bytes) -> bytes:
    bir = json.loads(bir_bytes)
    ctr = 0
    for fn in bir.get("functions", []):
        for bb in fn.get("blocks", []):
            out = []
            for inst in bb.get("instructions", []):
                si = inst.get("sync_info")
                waits = (si or {}).get("on_wait") or []
                if len(waits) > 1 and "engine" in inst:
                    for w in waits:
                        ctr += 1
                        out.append(
                            {
                                "debug": inst.get("debug", 0),
                                "engine": inst["engine"],
                                "ins": [],
                                "outs": [],
                                "name": f"{inst['name']}-sw{ctr}",
                                "opcode": "NoOp",
                                "sync_info": {"on_update": [], "on_wait": [w]},
                            }
                        )
                    si["on_wait"] = []
                out.append(inst)
            bb["instructions"] = out
    return json.dumps(bir).encode()


class _BassSplitWaits(bass.Bass):
    def to_json_bytes(self) -> bytes:
        return _split_multiwait_bir(super().to_json_bytes())


# ---------------------------------------------------------------------------
B = 2
S = 2048
D = 1024
HD = 64
N_CORES = 8
NHL = 4  # heads per core
E = NHL * HD  # 256
DT = D // 128  # 8
ST = S // 128  # 16
QBS = 512
NQB = S // QBS  # 4
F32 = mybir.dt.float32
F32R = mybir.dt.float32r
BF16 = mybir.dt.bfloat16
SCALE = 1.0 / np.sqrt(HD)


def build_nc() -> bass.Bass:
    nc = _BassSplitWaits()

    x_t = nc.dram_tensor("x_t", [D, S], F32R, kind="ExternalInput")
    wq_t = nc.dram_tensor("wq_t", [D, E], F32R, kind="ExternalInput")
    wk_t = nc.dram_tensor("wk_t", [D, E], F32R, kind="ExternalInput")
    wv_t = nc.dram_tensor("wv_t", [D, E], F32R, kind="ExternalInput")
    wo_t = nc.dram_tensor("wo_t", [E, D], F32R, kind="ExternalInput")
    tri_in = nc.dram_tensor("tri", [128, 128], F32R, kind="ExternalInput")
    ones_in = nc.dram_tensor("ones4", [128, NHL], F32R, kind="ExternalInput")
    out = nc.dram_tensor("out", [S, D], F32, kind="ExternalOutput")

    def dram_ap(t, base, ap):
        ref = t[0:1, 0:1]
        return bass.AP(tensor=ref.tensor, offset=base, ap=[list(a) for a in ap])

    with tile.TileContext(nc) as tc:
        with (
            tc.tile_pool(name="persist", bufs=1) as pp,
            tc.tile_pool(name="work", bufs=3) as wp,
            tc.tile_pool(name="ps", bufs=1, space="PSUM") as ps,
        ):
            # ---- mega tiles ----
            xm = pp.tile([128, DT * S], F32R, name="xm", tag="xm")
            xm3 = xm.rearrange("p (k s) -> p k s", k=DT)
            wqm = pp.tile([128, DT * E], F32R, name="wqm", tag="wqm")
            wqm3 = wqm.rearrange("p (k e) -> p k e", k=DT)
            wkm = pp.tile([128, DT * E], F32R, name="wkm", tag="wkm")
            wkm3 = wkm.rearrange("p (k e) -> p k e", k=DT)
            wvm = pp.tile([128, DT * E], F32R, name="wvm", tag="wvm")
            wvm3 = wvm.rearrange("p (k e) -> p k e", k=DT)
            wom = pp.tile([128, 2 * D], F32R, name="wom", tag="wom")
            wom3 = wom.rearrange("p (d c) -> p d c", d=2)
            tri = pp.tile([128, 128], F32R, name="tri", tag="tri")
            tri_bf = pp.tile([128, 128], BF16, name="tri_bf", tag="tri_bf")
            ones_col = pp.tile([128, NHL], F32R, name="ones_col", tag="ones_col")

            # ---- loads: few large strided DMAs, issued in first-use order
            # (HWDGE and the DMA engine pool are shared serial resources, so
            # issue order is transfer priority). x on Pool/SWDGE, weights on
            # SP/HWDGE, constants on the DVE queue.
            def x_dma(ktlo, kthi, nb):
                nc.gpsimd.dma_start(
                    out=xm3[:, ktlo:kthi, nb * QBS : (nb + 1) * QBS],
                    in_=dram_ap(
                        x_t,
                        ktlo * 128 * S + nb * QBS,
                        [[S, 128], [128 * S, kthi - ktlo], [1, QBS]],
                    ),
                )

            def w_dma(wdram, dst, half):
                nc.sync.dma_start(
                    out=dst[:, :, half * 128 : (half + 1) * 128],
                    in_=dram_ap(
                        wdram, half * 128, [[E, 128], [128 * E, DT], [1, 128]]
                    ),
                )

            nc.scalar.dma_start(out=tri, in_=tri_in[:, :])
            nc.scalar.dma_start(out=ones_col, in_=ones_in[:, :])
            x_dma(0, 2, 0)
            w_dma(wq_t, wqm3, 0)
            x_dma(2, 4, 0)
            w_dma(wk_t, wkm3, 0)
            x_dma(4, 6, 0)
            x_dma(6, 8, 0)
            nc.sync.dma_start(
                out=wvm3[:, :, :],
                in_=dram_ap(wv_t, 0, [[E, 128], [128 * E, DT], [1, E]]),
            )
            w_dma(wq_t, wqm3, 1)
            w_dma(wk_t, wkm3, 1)
            nc.vector.tensor_copy(out=tri_bf, in_=tri)
            nc.gpsimd.dma_start(
                out=xm3[:, :, QBS : 2 * QBS],
                in_=dram_ap(x_t, QBS, [[S, 128], [128 * S, DT], [1, QBS]]),
            )
            nc.sync.dma_start(
                out=wom3[:, :, :],
                in_=dram_ap(wo_t, 0, [[D, 128], [128 * D, 2], [1, D]]),
            )
            for nb in (2, 3):
                nc.gpsimd.dma_start(
                    out=xm3[:, :, nb * QBS : (nb + 1) * QBS],
                    in_=dram_ap(x_t, nb * QBS, [[S, 128], [128 * S, DT], [1, QBS]]),
                )

            # ---- persistent intermediates ----
            q_T = [pp.tile([128, S], F32R, name=f"qT{p}", tag=f"qT{p}") for p in range(2)]
            k_T = [pp.tile([128, S], F32R, name=f"kT{p}", tag=f"kT{p}") for p in range(2)]
            v_aug = [
                pp.tile([128, NHL * (HD + 1)], BF16, name=f"va{st}", tag=f"va{st}")
                for st in range(ST)
            ]
            ctx_T = [pp.tile([128, S], F32R, name=f"cT{p}", tag=f"cT{p}") for p in range(2)]

            # ---- unit builders: each unit is ~2 matmuls or one copy ----
            def qk_units(p, nb, wm3, dst):
                sl = slice(nb * QBS, (nb + 1) * QBS)
                cell = {}
                units = []

                def mk(j):
                    def u():
                        if j == 0:
                            cell["acc"] = ps.tile(
                                [128, QBS], F32, name="acc", tag="qk", bufs=2
                            )
                        for kt in (2 * j, 2 * j + 1):
                            nc.tensor.matmul(
                                cell["acc"],
                                lhsT=wm3[:, kt, p * 128 : (p + 1) * 128],
                                rhs=xm3[:, kt, sl],
                                start=(kt == 0),
                                stop=(kt == DT - 1),
                            )

                    return u

                units = [mk(j) for j in range(4)]

                def fin():
                    nc.vector.tensor_copy(out=dst[p][:, sl], in_=cell["acc"])

                units.append(fin)
                return units

            def v_units(st):
                cell = {}

                def mk(j):
                    def u():
                        if j == 0:
                            cell["acc"] = ps.tile(
                                [128, QBS], F32, name="acc", tag="qk", bufs=2
                            )
                        for kt in (2 * j, 2 * j + 1):
                            nc.tensor.matmul(
                                cell["acc"][:, 0:E],
                                lhsT=xm3[:, kt, st * 128 : (st + 1) * 128],
                                rhs=wvm3[:, kt, :],
                                start=(kt == 0),
                                stop=(kt == DT - 1),
                            )

                    return u

                units = [mk(j) for j in range(4)]

                def fin():
                    va = v_aug[st].rearrange("p (h c) -> p h c", h=NHL)
                    nc.vector.tensor_copy(
                        out=va[:, :, 0:HD],
                        in_=cell["acc"][:, 0:E].rearrange("p (h c) -> p h c", h=NHL),
                    )
                    nc.vector.tensor_copy(
                        out=va[:, :, HD : HD + 1],
                        in_=ones_col.rearrange("p (h c) -> p h c", c=1),
                    )

                units.append(fin)
                return units

            def outproj_units(st, tag="qk", direct=False, copy_eng=None):
                cell = {}

                def mk_mm(nb):
                    def u():
                        pso = ps.tile([128, QBS], F32, name="pso", tag=tag, bufs=2)
                        cell[nb] = pso
                        for dt_ in range(2):
                            nc.tensor.matmul(
                                pso,
                                lhsT=ctx_T[dt_][:, st * 128 : (st + 1) * 128],
                                rhs=wom3[:, dt_, nb * QBS : (nb + 1) * QBS],
                                start=(dt_ == 0),
                                stop=(dt_ == 1),
                            )

                    return u

                def mk_fin(nb):
                    def u():
                        if direct:
                            # kernel end: DMA straight from PSUM, skip staging
                            nc.sync.dma_start(
                                out=out[
                                    st * 128 : (st + 1) * 128,
                                    nb * QBS : (nb + 1) * QBS,
                                ],
                                in_=cell[nb],
                            )
                            return
                        # stage via SBUF (frees the PSUM slot fast) and DMA the
                        # half right away so the tail's last DMA chain is short
                        if nb == 0:
                            cell["osb"] = wp.tile(
                                [128, D], F32, name="osb", tag="osb", bufs=4
                            )
                        if copy_eng == "scalar":
                            nc.scalar.copy(
                                out=cell["osb"][:, nb * QBS : (nb + 1) * QBS],
                                in_=cell[nb],
                            )
                        else:
                            nc.vector.tensor_copy(
                                out=cell["osb"][:, nb * QBS : (nb + 1) * QBS],
                                in_=cell[nb],
                            )
                        nc.sync.dma_start(
                            out=out[st * 128 : (st + 1) * 128, nb * QBS : (nb + 1) * QBS],
                            in_=cell["osb"][:, nb * QBS : (nb + 1) * QBS],
                        )

                    return u

                return [mk_mm(0), mk_fin(0), mk_mm(1), mk_fin(1)]

            # ---- attention block with deferred normalization ----
            def attention(p, qb, fillers=(), last=False):
                fillers = list(fillers)
                n_kt = 4 * qb + 4
                ctxs = [
                    ps.tile([128, QBS], F32, name=f"ctx{h}", tag="ctx", bufs=2)
                    for h in range(2)
                ]
                pts = {}
                for kt in range(n_kt + 1):
                    if kt < n_kt:
                        o = 0 if kt < 4 * qb else (kt - 4 * qb) * 128
                        o_mm = min(o, QBS - 256)
                        s_ps = ps.tile([128, 2 * QBS], F32, name="s_ps", tag="s", bufs=2)
                        for hl in range(2):
                            nc.tensor.matmul(
                                s_ps[:, hl * QBS + o_mm : (hl + 1) * QBS],
                                lhsT=k_T[p][
                                    hl * HD : (hl + 1) * HD, kt * 128 : (kt + 1) * 128
                                ],
                                rhs=q_T[p][
                                    hl * HD : (hl + 1) * HD,
                                    qb * QBS + o_mm : (qb + 1) * QBS,
                                ],
                                start=True,
                                stop=True,
                            )
                        pt = wp.tile([128, 2 * QBS], BF16, name="pt", tag="pt", bufs=4)
                        sv = s_ps.rearrange("p (h q) -> p h q", h=2)
                        pv = pt.rearrange("p (h q) -> p h q", h=2)
                        nc.scalar.activation(
                            out=pv[:, :, o:QBS],
                            in_=sv[:, :, o:QBS],
                            func=mybir.ActivationFunctionType.Exp,
                            scale=float(SCALE),
                        )
                        if kt >= 4 * qb:
                            for hl in range(2):
                                blk = pt[:, hl * QBS + o : hl * QBS + o + 128]
                                nc.vector.tensor_mul(blk, blk, tri_bf)
                        pts[kt] = (pt, o)
                    if kt > 0:
                        pt, o = pts.pop(kt - 1)
                        for hl in range(2):
                            nc.tensor.matmul(
                                ctxs[hl][0 : HD + 1, o:QBS],
                                lhsT=v_aug[kt - 1][
                                    :, (2 * p + hl) * (HD + 1) : (2 * p + hl + 1) * (HD + 1)
                                ],
                                rhs=pt[:, hl * QBS + o : (hl + 1) * QBS],
                                start=(kt - 1 == 0),
                                stop=(kt - 1 == n_kt - 1),
                                skip_group_check=True,
                            )
                    # last block pops slowly so leftover units drain AFTER the
                    # cun/recip emission, hiding the normalize latency
                    for _ in range(1 if last else 2):
                        if fillers:
                            fillers.pop(0)()
                # stage unnormalized ctx through SBUF + reciprocal on the
                # denominator row; the broadcast+multiply is deferred
                cuns = []
                for hl in range(2):
                    cun = wp.tile([HD + 1, QBS], F32R, name="cun", tag="cun", bufs=4)
                    nc.vector.tensor_copy(out=cun, in_=ctxs[hl][0 : HD + 1, :])
                    # in-place reciprocal at partition 64 (equal in/out base —
                    # a DVE input at partition 64 with output at partition 0
                    # reads wrong data on HW)
                    with nc.allow_low_precision(reason="f32r is bitwise f32"):
                        nc.vector.reciprocal(
                            out=cun[HD : HD + 1, :], in_=cun[HD : HD + 1, :]
                        )
                    cuns.append(cun)
                while fillers:
                    fillers.pop(0)()

                def mk_norm(hl):
                    cun = cuns[hl]

                    def u():
                        # broadcast recip row across 64 partitions with a
                        # 1-row matmul: ones(1,64)^T @ recip(1,QBS)
                        bc = ps.tile([128, QBS], F32, name="bc", tag="qk", bufs=2)
                        # tri row 64 cols 64:128 is all-ones at partition 64,
                        # matching the recip row's base partition
                        nc.tensor.matmul(
                            bc[0:HD, :],
                            lhsT=tri[HD : HD + 1, HD : 2 * HD],
                            rhs=cun[HD : HD + 1, :],
                            start=True,
                            stop=True,
                        )
                        nc.vector.tensor_mul(
                            ctx_T[p][hl * HD : (hl + 1) * HD, qb * QBS : (qb + 1) * QBS],
                            cun[0:HD, :],
                            bc[0:HD, :],
                        )

                    return u

                norm = [mk_norm(0), mk_norm(1)]
                if last:
                    for u in norm:
                        u()
                    return []
                return norm

            def with_norm(units, norm):
                units = list(units)
                return units[:4] + list(norm) + units[4:]

            # ---- emission schedule ----
            for grp in (
                qk_units(0, 0, wqm3, q_T),
                qk_units(0, 0, wkm3, k_T),
            ):
                for u in grp:
                    u()
            for st in range(4):
                for u in v_units(st):
                    u()
            # att(0,0) gets fillers so its ACT-serial warmup doesn't stall PE;
            # qk(0,1) drains late enough that x block 1 (~22us) has landed
            n00 = attention(
                0, 0,
                qk_units(1, 0, wqm3, q_T) + qk_units(1, 0, wkm3, k_T)
                + qk_units(0, 1, wqm3, q_T),
            )
            # v(4..7) must be scheduled a block BEFORE att(0,1) reads them
            n10 = attention(
                1, 0,
                with_norm(
                    qk_units(0, 1, wkm3, k_T)
                    + v_units(4) + v_units(5) + v_units(6) + v_units(7),
                    n00,
                ),
            )
            n01 = attention(
                0, 1,
                with_norm(
                    qk_units(1, 1, wqm3, q_T) + qk_units(1, 1, wkm3, k_T), n10
                ),
            )
            n11 = attention(
                1, 1,
                with_norm(
                    qk_units(0, 2, wqm3, q_T) + qk_units(0, 2, wkm3, k_T)
                    + v_units(8) + v_units(9) + v_units(10) + v_units(11),
                    n01,
                ),
            )
            n02 = attention(
                0, 2,
                with_norm(
                    qk_units(1, 2, wqm3, q_T) + qk_units(1, 2, wkm3, k_T), n11
                ),
            )
            n12 = attention(
                1, 2,
                with_norm(
                    qk_units(0, 3, wqm3, q_T) + qk_units(0, 3, wkm3, k_T)
                    + v_units(12) + v_units(13) + v_units(14) + v_units(15)
                    + outproj_units(0) + outproj_units(1),
                    n02,
                ),
            )
            n03 = attention(
                0, 3,
                with_norm(
                    qk_units(1, 3, wqm3, q_T) + qk_units(1, 3, wkm3, k_T)
                    + outproj_units(2) + outproj_units(3)
                    + outproj_units(4) + outproj_units(5),
                    n12,
                ),
            )
            attention(
                1, 3,
                with_norm(
                    outproj_units(6) + outproj_units(7) + outproj_units(8)
                    + outproj_units(9) + outproj_units(10) + outproj_units(11),
                    n03,
                ),
                last=True,
            )
            # tail: alternate PSUM tags (ctx tag is free now) for a 4-slot
            # rotation; the last q-tile DMAs straight from PSUM
            for st in range(12, 16):
                for u in outproj_units(
                    st, tag=("qk" if st % 2 == 0 else "ctx"), copy_eng="scalar"
                ):
                    u()
    return nc


_NC_CACHE = {}


def _get_nc() -> bass.Bass:
    if "nc" not in _NC_CACHE:
        _NC_CACHE["nc"] = build_nc()
    return _NC_CACHE["nc"]


def kernel(in_features: np.ndarray, Wqkv: np.ndarray, Wo: np.ndarray) -> np.ndarray:
    in_features = np.ascontiguousarray(np.asarray(in_features, dtype=np.float32))
    Wqkv = np.asarray(Wqkv, dtype=np.float32)
    Wo = np.asarray(Wo, dtype=np.float32)

    tri = np.triu(np.ones((128, 128), dtype=np.float32))  # P^T[k,q] valid iff q >= k

    in_maps = []
    for c in range(N_CORES):
        b, g = divmod(c, NHL)
        sl = slice(g * E, (g + 1) * E)
        in_maps.append(
            {
                "x_t": np.ascontiguousarray(in_features[b].T),
                "wq_t": np.ascontiguousarray(Wqkv[sl, :].T),
                "wk_t": np.ascontiguousarray(Wqkv[D:][sl, :].T),
                "wv_t": np.ascontiguousarray(Wqkv[2 * D :][sl, :].T),
                "wo_t": np.ascontiguousarray(Wo[:, sl].T),
                "tri": tri,
                "ones4": np.ones((128, NHL), dtype=np.float32),
            }
        )

    res = run_bass_kernel_spmd(_get_nc(), in_maps, core_ids=list(range(N_CORES)))
    outs = [res.results[c]["out"] for c in range(N_CORES)]
    return np.stack(
        [outs[0] + outs[1] + outs[2] + outs[3], outs[4] + outs[5] + outs[6] + outs[7]],
        axis=0,
    )



# revision 2
# speedup vs baseline: 1.0417x; 1.0417x over previous
"""Causal MHSA Trainium2 kernel (8 NeuronCores) — v4.

Sharding: core c = 4*b + g handles batch b and head-group g (4 of 16
heads); host sums the 4 head-group partial projections per batch.

v4 (vs v3):
- All streaming tensors are bf16: x / Wq / Wk / Wv / Wo inputs arrive as
  host-prepared bf16 SBUF images (one strided DMA each, 2-4KB rows), and
  the output partial is written bf16 (host upcasts and sums). Total DMA
  drops from ~20MB to ~8MB per core, shrinking the startup window and the
  tail drain.
- q_T/k_T/ctx_T live in bf16, so the diagonal score matmuls no longer
  need >=256-col widening (bf16 runs 1 cycle/col at any width).
- Output staging is always through SBUF (bf16), never direct from PSUM.
"""

import json

import ml_dtypes
import numpy as np

import concourse.bass as bass
import concourse.mybir as mybir
import concourse.tile as tile
from concourse.bass_utils import run_bass_kernel_spmd

# ---------------------------------------------------------------------------
# Workaround: this container's walrus rejects instructions carrying more
# than one semaphore wait ("Too many sync wait commands", e.g. on the
# TileContext final drain). Split every multi-wait instruction into
# single-wait NoOps on the same engine placed immediately before it.
# ---------------------------------------------------------------------------


def _split_multiwait_bir(bir_bytes: bytes) -> bytes:
    bir = json.loads(bir_bytes)
    ctr = 0
    for fn in bir.get("functions", []):
        for bb in fn.get("blocks", []):
            out = []
            for inst in bb.get("instructions", []):
                si = inst.get("sync_info")
                waits = (si or {}).get("on_wait") or []
                if len(waits) > 1 and "engine" in inst:
                    for w in waits:
                        ctr += 1
                        out.append(
                            {
                                "debug": inst.get("debug", 0),
                                "engine": inst["engine"],
                                "ins": [],
                                "outs": [],
                                "name": f"{inst['name']}-sw{ctr}",
                                "opcode": "NoOp",
                                "sync_info": {"on_update": [], "on_wait": [w]},
                            }
                        )
                    si["on_wait"] = []
                out.append(inst)
            bb["instructions"] = out
    return json.dumps(bir).encode()


class _BassSplitWaits(bass.Bass):
    def to_json_bytes(self) -> bytes:
        return _split_multiwait_bir(super().to_json_bytes())


# ---------------------------------------------------------------------------
B = 2
S = 2048
D = 1024
HD = 64
N_CORES = 8
NHL = 4  # heads per core
E = NHL * HD  # 256
DT = D // 128  # 8
ST = S // 128  # 16
QBS = 512
NQB = S // QBS  # 4
F32 = mybir.dt.float32
F32R = mybir.dt.float32r
BF16 = mybir.dt.bfloat16
SCALE = 1.0 / np.sqrt(HD)


def build_nc() -> bass.Bass:
    nc = _BassSplitWaits()

    # host-prepared SBUF images (partition-major): x_img[p, kt*S+s],
    # w*_img[p, kt*E+e], wo_img[p, d*D+c]
    x_img = nc.dram_tensor("x_img", [128, DT * S], BF16, kind="ExternalInput")
    wq_img = nc.dram_tensor("wq_img", [128, DT * E], BF16, kind="ExternalInput")
    wk_img = nc.dram_tensor("wk_img", [128, DT * E], BF16, kind="ExternalInput")
    wv_img = nc.dram_tensor("wv_img", [128, DT * E], BF16, kind="ExternalInput")
    wo_img = nc.dram_tensor("wo_img", [128, 2 * D], BF16, kind="ExternalInput")
    tri_in = nc.dram_tensor("tri", [128, 128], F32R, kind="ExternalInput")
    ones_in = nc.dram_tensor("ones4", [128, NHL], F32R, kind="ExternalInput")
    out = nc.dram_tensor("out", [S, D], BF16, kind="ExternalOutput")

    def dram_ap(t, base, ap):
        ref = t[0:1, 0:1]
        return bass.AP(tensor=ref.tensor, offset=base, ap=[list(a) for a in ap])

    with tile.TileContext(nc) as tc:
        with (
            tc.tile_pool(name="persist", bufs=1) as pp,
            tc.tile_pool(name="work", bufs=3) as wp,
            tc.tile_pool(name="ps", bufs=1, space="PSUM") as ps,
        ):
            # ---- mega tiles ----
            xm = pp.tile([128, DT * S], BF16, name="xm", tag="xm")
            xm3 = xm.rearrange("p (k s) -> p k s", k=DT)
            wqm = pp.tile([128, DT * E], BF16, name="wqm", tag="wqm")
            wqm3 = wqm.rearrange("p (k e) -> p k e", k=DT)
            wkm = pp.tile([128, DT * E], BF16, name="wkm", tag="wkm")
            wkm3 = wkm.rearrange("p (k e) -> p k e", k=DT)
            wvm = pp.tile([128, DT * E], BF16, name="wvm", tag="wvm")
            wvm3 = wvm.rearrange("p (k e) -> p k e", k=DT)
            wom = pp.tile([128, 2 * D], BF16, name="wom", tag="wom")
            wom3 = wom.rearrange("p (d c) -> p d c", d=2)
            tri = pp.tile([128, 128], F32R, name="tri", tag="tri")
            tri_bf = pp.tile([128, 128], BF16, name="tri_bf", tag="tri_bf")
            ones_col = pp.tile([128, NHL], F32R, name="ones_col", tag="ones_col")

            # ---- loads: strided DMAs straight off the host images, in
            # first-use order. x on Pool/SWDGE, weights on SP/HWDGE,
            # constants on the ACT queue.
            def x_dma(ktlo, kthi, slo, shi):
                nc.gpsimd.dma_start(
                    out=xm3[:, ktlo:kthi, slo:shi],
                    in_=dram_ap(
                        x_img,
                        ktlo * S + slo,
                        [[DT * S, 128], [S, kthi - ktlo], [1, shi - slo]],
                    ),
                )

            def w_dma(wdram, dst, ktlo, kthi):
                nc.sync.dma_start(
                    out=dst[:, ktlo:kthi, :],
                    in_=dram_ap(
                        wdram, ktlo * E, [[DT * E, 128], [E, kthi - ktlo], [1, E]]
                    ),
                )

            nc.scalar.dma_start(out=tri, in_=tri_in[:, :])
            nc.scalar.dma_start(out=ones_col, in_=ones_in[:, :])
            w_dma(wq_img, wqm3, 0, 4)
            x_dma(0, 4, 0, QBS)
            w_dma(wq_img, wqm3, 4, 8)
            x_dma(4, 8, 0, QBS)
            w_dma(wk_img, wkm3, 0, 8)
            nc.vector.tensor_copy(out=tri_bf, in_=tri)
            x_dma(0, 8, QBS, 2 * QBS)
            nc.sync.dma_start(
                out=wvm3[:, :, :],
                in_=dram_ap(wv_img, 0, [[DT * E, 128], [1, DT * E]]),
            )
            x_dma(0, 8, 2 * QBS, 3 * QBS)
            nc.sync.dma_start(
                out=wom3[:, :, :],
                in_=dram_ap(wo_img, 0, [[2 * D, 128], [1, 2 * D]]),
            )
            x_dma(0, 8, 3 * QBS, 4 * QBS)

            # ---- persistent intermediates ----
            q_T = [pp.tile([128, S], BF16, name=f"qT{p}", tag=f"qT{p}") for p in range(2)]
            k_T = [pp.tile([128, S], BF16, name=f"kT{p}", tag=f"kT{p}") for p in range(2)]
            v_aug = [
                pp.tile([128, NHL * (HD + 1)], BF16, name=f"va{st}", tag=f"va{st}")
                for st in range(ST)
            ]
            ctx_T = [pp.tile([128, S], BF16, name=f"cT{p}", tag=f"cT{p}") for p in range(2)]

            # ---- unit builders: each unit is ~2 matmuls or one copy ----
            def qk_units(p, nb, wm3, dst):
                sl = slice(nb * QBS, (nb + 1) * QBS)
                cell = {}
                units = []

                def mk(j):
                    def u():
                        if j == 0:
                            cell["acc"] = ps.tile(
                                [128, QBS], F32, name="acc", tag="qk", bufs=2
                            )
                        for kt in (2 * j, 2 * j + 1):
                            nc.tensor.matmul(
                                cell["acc"],
                                lhsT=wm3[:, kt, p * 128 : (p + 1) * 128],
                                rhs=xm3[:, kt, sl],
                                start=(kt == 0),
                                stop=(kt == DT - 1),
                            )

                    return u

                units = [mk(j) for j in range(4)]

                def fin():
                    nc.vector.tensor_copy(out=dst[p][:, sl], in_=cell["acc"])

                units.append(fin)
                return units

            def v_units(st):
                cell = {}

                def mk(j):
                    def u():
                        if j == 0:
                            cell["acc"] = ps.tile(
                                [128, QBS], F32, name="acc", tag="qk", bufs=2
                            )
                        for kt in (2 * j, 2 * j + 1):
                            nc.tensor.matmul(
                                cell["acc"][:, 0:E],
                                lhsT=xm3[:, kt, st * 128 : (st + 1) * 128],
                                rhs=wvm3[:, kt, :],
                                start=(kt == 0),
                                stop=(kt == DT - 1),
                            )

                    return u

                units = [mk(j) for j in range(4)]

                def fin():
                    va = v_aug[st].rearrange("p (h c) -> p h c", h=NHL)
                    nc.vector.tensor_copy(
                        out=va[:, :, 0:HD],
                        in_=cell["acc"][:, 0:E].rearrange("p (h c) -> p h c", h=NHL),
                    )
                    nc.vector.tensor_copy(
                        out=va[:, :, HD : HD + 1],
                        in_=ones_col.rearrange("p (h c) -> p h c", c=1),
                    )

                units.append(fin)
                return units

            def outproj_units(st, tag="qk", copy_eng=None):
                cell = {}

                def mk_mm(nb):
                    def u():
                        pso = ps.tile([128, QBS], F32, name="pso", tag=tag, bufs=2)
                        cell[nb] = pso
                        for dt_ in range(2):
                            nc.tensor.matmul(
                                pso,
                                lhsT=ctx_T[dt_][:, st * 128 : (st + 1) * 128],
                                rhs=wom3[:, dt_, nb * QBS : (nb + 1) * QBS],
                                start=(dt_ == 0),
                                stop=(dt_ == 1),
                            )

                    return u

                def mk_fin(nb):
                    def u():
                        # stage via SBUF bf16 (frees the PSUM slot fast) and
                        # DMA the half right away so the tail's last DMA
                        # chain is short
                        if nb == 0:
                            cell["osb"] = wp.tile(
                                [128, D], BF16, name="osb", tag="osb", bufs=4
                            )
                        if copy_eng == "scalar":
                            nc.scalar.copy(
                                out=cell["osb"][:, nb * QBS : (nb + 1) * QBS],
                                in_=cell[nb],
                            )
                        else:
                            nc.vector.tensor_copy(
                                out=cell["osb"][:, nb * QBS : (nb + 1) * QBS],
                                in_=cell[nb],
                            )
                        nc.sync.dma_start(
                            out=out[st * 128 : (st + 1) * 128, nb * QBS : (nb + 1) * QBS],
                            in_=cell["osb"][:, nb * QBS : (nb + 1) * QBS],
                        )

                    return u

                return [mk_mm(0), mk_fin(0), mk_mm(1), mk_fin(1)]

            # ---- attention block with deferred normalization ----
            def attention(p, qb, fillers=(), last=False):
                fillers = list(fillers)
                n_kt = 4 * qb + 4
                ctxs = [
                    ps.tile([128, QBS], F32, name=f"ctx{h}", tag="ctx", bufs=2)
                    for h in range(2)
                ]
                pts = {}
                for kt in range(n_kt + 1):
                    if kt < n_kt:
                        o = 0 if kt < 4 * qb else (kt - 4 * qb) * 128
                        s_ps = ps.tile([128, 2 * QBS], F32, name="s_ps", tag="s", bufs=2)
                        for hl in range(2):
                            nc.tensor.matmul(
                                s_ps[:, hl * QBS + o : (hl + 1) * QBS],
                                lhsT=k_T[p][
                                    hl * HD : (hl + 1) * HD, kt * 128 : (kt + 1) * 128
                                ],
                                rhs=q_T[p][
                                    hl * HD : (hl + 1) * HD,
                                    qb * QBS + o : (qb + 1) * QBS,
                                ],
                                start=True,
                                stop=True,
                            )
                        pt = wp.tile([128, 2 * QBS], BF16, name="pt", tag="pt", bufs=4)
                        sv = s_ps.rearrange("p (h q) -> p h q", h=2)
                        pv = pt.rearrange("p (h q) -> p h q", h=2)
                        nc.scalar.activation(
                            out=pv[:, :, o:QBS],
                            in_=sv[:, :, o:QBS],
                            func=mybir.ActivationFunctionType.Exp,
                            scale=float(SCALE),
                        )
                        if kt >= 4 * qb:
                            for hl in range(2):
                                blk = pt[:, hl * QBS + o : hl * QBS + o + 128]
                                nc.vector.tensor_mul(blk, blk, tri_bf)
                        pts[kt] = (pt, o)
                    if kt > 0:
                        pt, o = pts.pop(kt - 1)
                        for hl in range(2):
                            nc.tensor.matmul(
                                ctxs[hl][0 : HD + 1, o:QBS],
                                lhsT=v_aug[kt - 1][
                                    :, (2 * p + hl) * (HD + 1) : (2 * p + hl + 1) * (HD + 1)
                                ],
                                rhs=pt[:, hl * QBS + o : (hl + 1) * QBS],
                                start=(kt - 1 == 0),
                                stop=(kt - 1 == n_kt - 1),
                                skip_group_check=True,
                            )
                    # last block pops slowly so leftover units drain AFTER the
                    # cun/recip emission, hiding the normalize latency
                    for _ in range(1 if last else 2):
                        if fillers:
                            fillers.pop(0)()
                # stage unnormalized ctx through SBUF + reciprocal on the
                # denominator row; the broadcast+multiply is deferred
                cuns = []
                for hl in range(2):
                    cun = wp.tile([HD + 1, QBS], F32R, name="cun", tag="cun", bufs=4)
                    nc.vector.tensor_copy(out=cun, in_=ctxs[hl][0 : HD + 1, :])
                    # in-place reciprocal at partition 64 (equal in/out base —
                    # a DVE input at partition 64 with output at partition 0
                    # reads wrong data on HW)
                    with nc.allow_low_precision(reason="f32r is bitwise f32"):
                        nc.vector.reciprocal(
                            out=cun[HD : HD + 1, :], in_=cun[HD : HD + 1, :]
                        )
                    cuns.append(cun)
                while fillers:
                    fillers.pop(0)()

                def mk_norm(hl):
                    cun = cuns[hl]

                    def u():
                        # broadcast recip row across 64 partitions with a
                        # 1-row matmul: ones(1,64)^T @ recip(1,QBS)
                        bc = ps.tile([128, QBS], F32, name="bc", tag="qk", bufs=2)
                        # tri row 64 cols 64:128 is all-ones at partition 64,
                        # matching the recip row's base partition
                        nc.tensor.matmul(
                            bc[0:HD, :],
                            lhsT=tri[HD : HD + 1, HD : 2 * HD],
                            rhs=cun[HD : HD + 1, :],
                            start=True,
                            stop=True,
                        )
                        nc.vector.tensor_mul(
                            ctx_T[p][hl * HD : (hl + 1) * HD, qb * QBS : (qb + 1) * QBS],
                            cun[0:HD, :],
                            bc[0:HD, :],
                        )

                    return u

                norm = [mk_norm(0), mk_norm(1)]
                if last:
                    for u in norm:
                        u()
                    return []
                return norm

            def with_norm(units, norm):
                units = list(units)
                return units[:4] + list(norm) + units[4:]

            # ---- emission schedule ----
            for grp in (
                qk_units(0, 0, wqm3, q_T),
                qk_units(0, 0, wkm3, k_T),
            ):
                for u in grp:
                    u()
            for st in range(4):
                for u in v_units(st):
                    u()
            # att(0,0) gets fillers so its ACT-serial warmup doesn't stall PE;
            # qk(0,1) drains late enough that x block 1 has landed
            n00 = attention(
                0, 0,
                qk_units(1, 0, wqm3, q_T) + qk_units(1, 0, wkm3, k_T)
                + qk_units(0, 1, wqm3, q_T),
            )
            # v(4..7) must be scheduled a block BEFORE att(0,1) reads them
            n10 = attention(
                1, 0,
                with_norm(
                    qk_units(0, 1, wkm3, k_T)
                    + v_units(4) + v_units(5) + v_units(6) + v_units(7),
                    n00,
                ),
            )
            n01 = attention(
                0, 1,
                with_norm(
                    qk_units(1, 1, wqm3, q_T) + qk_units(1, 1, wkm3, k_T), n10
                ),
            )
            n11 = attention(
                1, 1,
                with_norm(
                    qk_units(0, 2, wqm3, q_T) + qk_units(0, 2, wkm3, k_T)
                    + v_units(8) + v_units(9) + v_units(10) + v_units(11),
                    n01,
                ),
            )
            n02 = attention(
                0, 2,
                with_norm(
                    qk_units(1, 2, wqm3, q_T) + qk_units(1, 2, wkm3, k_T), n11
                ),
            )
            n12 = attention(
                1, 2,
                with_norm(
                    qk_units(0, 3, wqm3, q_T) + qk_units(0, 3, wkm3, k_T)
                    + v_units(12) + v_units(13) + v_units(14) + v_units(15)
                    + outproj_units(0) + outproj_units(1),
                    n02,
                ),
            )
            n03 = attention(
                0, 3,
                with_norm(
                    qk_units(1, 3, wqm3, q_T) + qk_units(1, 3, wkm3, k_T)
                    + outproj_units(2) + outproj_units(3)
                    + outproj_units(4) + outproj_units(5),
                    n12,
                ),
            )
            attention(
                1, 3,
                with_norm(
                    outproj_units(6) + outproj_units(7) + outproj_units(8)
                    + outproj_units(9) + outproj_units(10) + outproj_units(11),
                    n03,
                ),
                last=True,
            )
            # tail: alternate PSUM tags (ctx tag is free now) for a 4-slot
            # rotation
            for st in range(12, 16):
                for u in outproj_units(
                    st, tag=("qk" if st % 2 == 0 else "ctx"), copy_eng="scalar"
                ):
                    u()
    return nc


_NC_CACHE = {}


def _get_nc() -> bass.Bass:
    if "nc" not in _NC_CACHE:
        _NC_CACHE["nc"] = build_nc()
    return _NC_CACHE["nc"]


def kernel(in_features: np.ndarray, Wqkv: np.ndarray, Wo: np.ndarray) -> np.ndarray:
    BF = ml_dtypes.bfloat16
    x32 = np.ascontiguousarray(np.asarray(in_features, dtype=np.float32))
    Wqkv = np.asarray(Wqkv, dtype=np.float32)
    Wo = np.asarray(Wo, dtype=np.float32)

    tri = np.triu(np.ones((128, 128), dtype=np.float32))  # P^T[k,q] valid iff q >= k

    def img_kpm(arr_t, k, f):
        # arr_t: [k*128, f] -> partition-major image [128, k*f]
        return np.ascontiguousarray(
            arr_t.reshape(k, 128, f).transpose(1, 0, 2).reshape(128, k * f).astype(BF)
        )

    in_maps = []
    for c in range(N_CORES):
        b, g = divmod(c, NHL)
        sl = slice(g * E, (g + 1) * E)
        in_maps.append(
            {
                "x_img": img_kpm(x32[b].T, DT, S),
                "wq_img": img_kpm(np.ascontiguousarray(Wqkv[sl, :]).T, DT, E),
                "wk_img": img_kpm(np.ascontiguousarray(Wqkv[D:][sl, :]).T, DT, E),
                "wv_img": img_kpm(np.ascontiguousarray(Wqkv[2 * D :][sl, :]).T, DT, E),
                "wo_img": img_kpm(np.ascontiguousarray(Wo[:, sl]).T, 2, D),
                "tri": tri,
                "ones4": np.ones((128, NHL), dtype=np.float32),
            }
        )

    res = run_bass_kernel_spmd(_get_nc(), in_maps, core_ids=list(range(N_CORES)))
    outs = [res.results[c]["out"].astype(np.float32) for c in range(N_CORES)]
    return np.stack(
        [outs[0] + outs[1] + outs[2] + outs[3], outs[4] + outs[5] + outs[6] + outs[7]],
        axis=0,
    )


# revision 3
# speedup vs baseline: 1.0907x; 1.0470x over previous
"""Causal MHSA Trainium2 kernel (8 NeuronCores) — v4.

Sharding: core c = 4*b + g handles batch b and head-group g (4 of 16
heads); host sums the 4 head-group partial projections per batch.

v4 (vs v3):
- All streaming tensors are bf16: x / Wq / Wk / Wv / Wo inputs arrive as
  host-prepared bf16 SBUF images (one strided DMA each, 2-4KB rows), and
  the output partial is written bf16 (host upcasts and sums). Total DMA
  drops from ~20MB to ~8MB per core, shrinking the startup window and the
  tail drain.
- q_T/k_T/ctx_T live in bf16, so the diagonal score matmuls no longer
  need >=256-col widening (bf16 runs 1 cycle/col at any width).
- Output staging is always through SBUF (bf16), never direct from PSUM.
"""

import json

import ml_dtypes
import numpy as np

import concourse.bass as bass
import concourse.mybir as mybir
import concourse.tile as tile
from concourse.bass_utils import run_bass_kernel_spmd

# ---------------------------------------------------------------------------
# Workaround: this container's walrus rejects instructions carrying more
# than one semaphore wait ("Too many sync wait commands", e.g. on the
# TileContext final drain). Split every multi-wait instruction into
# single-wait NoOps on the same engine placed immediately before it.
# ---------------------------------------------------------------------------


def _split_multiwait_bir(bir_bytes: bytes) -> bytes:
    bir = json.loads(bir_bytes)
    ctr = 0
    for fn in bir.get("functions", []):
        for bb in fn.get("blocks", []):
            out = []
            for inst in bb.get("instructions", []):
                si = inst.get("sync_info")
                waits = (si or {}).get("on_wait") or []
                if len(waits) > 1 and "engine" in inst:
                    for w in waits:
                        ctr += 1
                        out.append(
                            {
                                "debug": inst.get("debug", 0),
                                "engine": inst["engine"],
                                "ins": [],
                                "outs": [],
                                "name": f"{inst['name']}-sw{ctr}",
                                "opcode": "NoOp",
                                "sync_info": {"on_update": [], "on_wait": [w]},
                            }
                        )
                    si["on_wait"] = []
                out.append(inst)
            bb["instructions"] = out
    return json.dumps(bir).encode()


class _BassSplitWaits(bass.Bass):
    def to_json_bytes(self) -> bytes:
        return _split_multiwait_bir(super().to_json_bytes())


# ---------------------------------------------------------------------------
B = 2
S = 2048
D = 1024
HD = 64
N_CORES = 8
NHL = 4  # heads per core
E = NHL * HD  # 256
DT = D // 128  # 8
ST = S // 128  # 16
QBS = 512
NQB = S // QBS  # 4
F32 = mybir.dt.float32
F32R = mybir.dt.float32r
BF16 = mybir.dt.bfloat16
SCALE = 1.0 / np.sqrt(HD)


def build_nc() -> bass.Bass:
    nc = _BassSplitWaits()

    # host-prepared SBUF images (partition-major): x_img[p, kt*S+s],
    # w*_img[p, kt*E+e], wo_img[p, d*D+c]
    x_img = nc.dram_tensor("x_img", [128, DT * S], BF16, kind="ExternalInput")
    wq_img = nc.dram_tensor("wq_img", [128, DT * E], BF16, kind="ExternalInput")
    wk_img = nc.dram_tensor("wk_img", [128, DT * E], BF16, kind="ExternalInput")
    wv_img = nc.dram_tensor("wv_img", [128, DT * E], BF16, kind="ExternalInput")
    wo_img = nc.dram_tensor("wo_img", [128, 2 * D], BF16, kind="ExternalInput")
    tri_in = nc.dram_tensor("tri", [128, 128], F32R, kind="ExternalInput")
    ones_in = nc.dram_tensor("ones4", [128, NHL], F32R, kind="ExternalInput")
    out = nc.dram_tensor("out", [S, D], BF16, kind="ExternalOutput")

    def dram_ap(t, base, ap):
        ref = t[0:1, 0:1]
        return bass.AP(tensor=ref.tensor, offset=base, ap=[list(a) for a in ap])

    with tile.TileContext(nc) as tc:
        with (
            tc.tile_pool(name="persist", bufs=1) as pp,
            tc.tile_pool(name="work", bufs=3) as wp,
            tc.tile_pool(name="ps", bufs=1, space="PSUM") as ps,
        ):
            # ---- mega tiles ----
            xm = pp.tile([128, DT * S], BF16, name="xm", tag="xm")
            xm3 = xm.rearrange("p (k s) -> p k s", k=DT)
            wqm = pp.tile([128, DT * E], BF16, name="wqm", tag="wqm")
            wqm3 = wqm.rearrange("p (k e) -> p k e", k=DT)
            wkm = pp.tile([128, DT * E], BF16, name="wkm", tag="wkm")
            wkm3 = wkm.rearrange("p (k e) -> p k e", k=DT)
            wvm = pp.tile([128, DT * E], BF16, name="wvm", tag="wvm")
            wvm3 = wvm.rearrange("p (k e) -> p k e", k=DT)
            wom = pp.tile([128, 2 * D], BF16, name="wom", tag="wom")
            wom3 = wom.rearrange("p (d c) -> p d c", d=2)
            tri = pp.tile([128, 128], F32R, name="tri", tag="tri")
            tri_bf = pp.tile([128, 128], BF16, name="tri_bf", tag="tri_bf")
            ones_col = pp.tile([128, NHL], F32R, name="ones_col", tag="ones_col")

            # ---- loads: strided DMAs straight off the host images, in
            # first-use order. x on Pool/SWDGE, weights on SP/HWDGE,
            # constants on the ACT queue.
            def x_dma(ktlo, kthi, slo, shi):
                nc.gpsimd.dma_start(
                    out=xm3[:, ktlo:kthi, slo:shi],
                    in_=dram_ap(
                        x_img,
                        ktlo * S + slo,
                        [[DT * S, 128], [S, kthi - ktlo], [1, shi - slo]],
                    ),
                )

            def w_dma(wdram, dst, ktlo, kthi):
                nc.sync.dma_start(
                    out=dst[:, ktlo:kthi, :],
                    in_=dram_ap(
                        wdram, ktlo * E, [[DT * E, 128], [E, kthi - ktlo], [1, E]]
                    ),
                )

            w_dma(wq_img, wqm3, 0, 2)
            x_dma(0, 2, 0, QBS)
            nc.scalar.dma_start(out=tri, in_=tri_in[:, :])
            nc.scalar.dma_start(out=ones_col, in_=ones_in[:, :])
            w_dma(wq_img, wqm3, 2, 4)
            x_dma(2, 4, 0, QBS)
            w_dma(wq_img, wqm3, 4, 8)
            x_dma(4, 8, 0, QBS)
            w_dma(wk_img, wkm3, 0, 8)
            nc.vector.tensor_copy(out=tri_bf, in_=tri)
            nc.sync.dma_start(
                out=wvm3[:, :, :],
                in_=dram_ap(wv_img, 0, [[DT * E, 128], [1, DT * E]]),
            )
            x_dma(0, 8, QBS, 2 * QBS)
            x_dma(0, 8, 2 * QBS, 3 * QBS)
            nc.sync.dma_start(
                out=wom3[:, :, :],
                in_=dram_ap(wo_img, 0, [[2 * D, 128], [1, 2 * D]]),
            )
            x_dma(0, 8, 3 * QBS, 4 * QBS)

            # ---- persistent intermediates ----
            q_T = [pp.tile([128, S], BF16, name=f"qT{p}", tag=f"qT{p}") for p in range(2)]
            k_T = [pp.tile([128, S], BF16, name=f"kT{p}", tag=f"kT{p}") for p in range(2)]
            v_aug = [
                pp.tile([128, NHL * (HD + 1)], BF16, name=f"va{st}", tag=f"va{st}")
                for st in range(ST)
            ]
            ctx_T = [pp.tile([128, S], BF16, name=f"cT{p}", tag=f"cT{p}") for p in range(2)]

            # ---- unit builders: each unit is ~2 matmuls or one copy ----
            def qk_units(p, nb, wm3, dst):
                sl = slice(nb * QBS, (nb + 1) * QBS)
                cell = {}
                units = []

                def mk(j):
                    def u():
                        if j == 0:
                            cell["acc"] = ps.tile(
                                [128, QBS], F32, name="acc", tag="qk", bufs=2
                            )
                        for kt in (2 * j, 2 * j + 1):
                            nc.tensor.matmul(
                                cell["acc"],
                                lhsT=wm3[:, kt, p * 128 : (p + 1) * 128],
                                rhs=xm3[:, kt, sl],
                                start=(kt == 0),
                                stop=(kt == DT - 1),
                            )

                    return u

                units = [mk(j) for j in range(4)]

                def fin():
                    nc.vector.tensor_copy(out=dst[p][:, sl], in_=cell["acc"])

                units.append(fin)
                return units

            def v_units(st):
                cell = {}

                def mk(j):
                    def u():
                        if j == 0:
                            cell["acc"] = ps.tile(
                                [128, QBS], F32, name="acc", tag="qk", bufs=2
                            )
                        for kt in (2 * j, 2 * j + 1):
                            nc.tensor.matmul(
                                cell["acc"][:, 0:E],
                                lhsT=xm3[:, kt, st * 128 : (st + 1) * 128],
                                rhs=wvm3[:, kt, :],
                                start=(kt == 0),
                                stop=(kt == DT - 1),
                            )

                    return u

                units = [mk(j) for j in range(4)]

                def fin():
                    va = v_aug[st].rearrange("p (h c) -> p h c", h=NHL)
                    nc.vector.tensor_copy(
                        out=va[:, :, 0:HD],
                        in_=cell["acc"][:, 0:E].rearrange("p (h c) -> p h c", h=NHL),
                    )
                    nc.vector.tensor_copy(
                        out=va[:, :, HD : HD + 1],
                        in_=ones_col.rearrange("p (h c) -> p h c", c=1),
                    )

                units.append(fin)
                return units

            def outproj_units(st, tag="qk", copy_eng=None, tail=False):
                cell = {}

                def mk_mm(nb):
                    def u():
                        pso = ps.tile([128, QBS], F32, name="pso", tag=tag, bufs=2)
                        cell[nb] = pso
                        for dt_ in range(2):
                            nc.tensor.matmul(
                                pso,
                                lhsT=ctx_T[dt_][:, st * 128 : (st + 1) * 128],
                                rhs=wom3[:, dt_, nb * QBS : (nb + 1) * QBS],
                                start=(dt_ == 0),
                                stop=(dt_ == 1),
                            )

                    return u

                def mk_fin(nb, eng):
                    def u():
                        # stage via SBUF bf16 (frees the PSUM slot fast) and
                        # DMA the half right away so the tail's last DMA
                        # chain is short
                        if "osb" not in cell:
                            cell["osb"] = wp.tile(
                                [128, D], BF16, name="osb", tag="osb", bufs=4
                            )
                        if eng == "scalar":
                            nc.scalar.copy(
                                out=cell["osb"][:, nb * QBS : (nb + 1) * QBS],
                                in_=cell[nb],
                            )
                        else:
                            nc.vector.tensor_copy(
                                out=cell["osb"][:, nb * QBS : (nb + 1) * QBS],
                                in_=cell[nb],
                            )
                        # tail: odd-nb DMAs go out the SWDGE (Pool) queue so
                        # the 625ns/DMA HWDGE generation chain halves
                        dma_q = nc.gpsimd if (tail and nb == 1) else nc.sync
                        dma_q.dma_start(
                            out=out[st * 128 : (st + 1) * 128, nb * QBS : (nb + 1) * QBS],
                            in_=cell["osb"][:, nb * QBS : (nb + 1) * QBS],
                        )

                    return u

                if tail:
                    # both matmuls back-to-back (alternating PSUM tags give 4
                    # slots), staging copies split across ACT and DVE
                    return [
                        mk_mm(0),
                        mk_mm(1),
                        mk_fin(0, "scalar"),
                        mk_fin(1, "vector"),
                    ]
                return [mk_mm(0), mk_fin(0, copy_eng), mk_mm(1), mk_fin(1, copy_eng)]

            # ---- attention block with deferred normalization ----
            def attention(p, qb, fillers=(), last=False):
                fillers = list(fillers)
                n_kt = 4 * qb + 4
                ctxs = [
                    ps.tile([128, QBS], F32, name=f"ctx{h}", tag="ctx", bufs=2)
                    for h in range(2)
                ]
                pts = {}
                for kt in range(n_kt + 1):
                    if kt < n_kt:
                        o = 0 if kt < 4 * qb else (kt - 4 * qb) * 128
                        s_ps = ps.tile([128, 2 * QBS], F32, name="s_ps", tag="s", bufs=2)
                        for hl in range(2):
                            nc.tensor.matmul(
                                s_ps[:, hl * QBS + o : (hl + 1) * QBS],
                                lhsT=k_T[p][
                                    hl * HD : (hl + 1) * HD, kt * 128 : (kt + 1) * 128
                                ],
                                rhs=q_T[p][
                                    hl * HD : (hl + 1) * HD,
                                    qb * QBS + o : (qb + 1) * QBS,
                                ],
                                start=True,
                                stop=True,
                            )
                        pt = wp.tile([128, 2 * QBS], BF16, name="pt", tag="pt", bufs=4)
                        sv = s_ps.rearrange("p (h q) -> p h q", h=2)
                        pv = pt.rearrange("p (h q) -> p h q", h=2)
                        nc.scalar.activation(
                            out=pv[:, :, o:QBS],
                            in_=sv[:, :, o:QBS],
                            func=mybir.ActivationFunctionType.Exp,
                            scale=float(SCALE),
                        )
                        if kt >= 4 * qb:
                            for hl in range(2):
                                blk = pt[:, hl * QBS + o : hl * QBS + o + 128]
                                nc.vector.tensor_mul(blk, blk, tri_bf)
                        pts[kt] = (pt, o)
                    # fillers BEFORE ctx(kt-1): PE executes in order, so the
                    # (independent) fillers run while exp(kt-1) finishes; the
                    # ctx matmul then starts without exposing the ACT latency.
                    # Pops adapt so the filler list drains evenly across the
                    # block instead of leaving a burst stuck behind the last
                    # (dependency-carrying) ctx matmul.
                    iters_left = n_kt + 1 - kt
                    if last:
                        npop = 1
                    else:
                        npop = max(2, -(-len(fillers) // iters_left))
                    for _ in range(npop):
                        if fillers:
                            fillers.pop(0)()
                    if kt > 0:
                        pt, o = pts.pop(kt - 1)
                        for hl in range(2):
                            nc.tensor.matmul(
                                ctxs[hl][0 : HD + 1, o:QBS],
                                lhsT=v_aug[kt - 1][
                                    :, (2 * p + hl) * (HD + 1) : (2 * p + hl + 1) * (HD + 1)
                                ],
                                rhs=pt[:, hl * QBS + o : (hl + 1) * QBS],
                                start=(kt - 1 == 0),
                                stop=(kt - 1 == n_kt - 1),
                                skip_group_check=True,
                            )
                # stage unnormalized ctx through SBUF + reciprocal on the
                # denominator row; the broadcast+multiply is deferred
                cuns = []
                for hl in range(2):
                    cun = wp.tile([HD + 1, QBS], F32R, name="cun", tag="cun", bufs=4)
                    if last:
                        # keep the tail's DVE budget for recips/norm muls and
                        # staging copies; ACT has no exps left here
                        nc.scalar.copy(out=cun, in_=ctxs[hl][0 : HD + 1, :])
                    else:
                        nc.vector.tensor_copy(out=cun, in_=ctxs[hl][0 : HD + 1, :])
                    # in-place reciprocal at partition 64 (equal in/out base —
                    # a DVE input at partition 64 with output at partition 0
                    # reads wrong data on HW)
                    with nc.allow_low_precision(reason="f32r is bitwise f32"):
                        nc.vector.reciprocal(
                            out=cun[HD : HD + 1, :], in_=cun[HD : HD + 1, :]
                        )
                    cuns.append(cun)
                while fillers:
                    fillers.pop(0)()

                def mk_norm(hl):
                    cun = cuns[hl]

                    def u():
                        # broadcast recip row across 64 partitions with a
                        # 1-row matmul: ones(1,64)^T @ recip(1,QBS)
                        bc = ps.tile([128, QBS], F32, name="bc", tag="qk", bufs=2)
                        # tri row 64 cols 64:128 is all-ones at partition 64,
                        # matching the recip row's base partition
                        nc.tensor.matmul(
                            bc[0:HD, :],
                            lhsT=tri[HD : HD + 1, HD : 2 * HD],
                            rhs=cun[HD : HD + 1, :],
                            start=True,
                            stop=True,
                        )
                        nc.vector.tensor_mul(
                            ctx_T[p][hl * HD : (hl + 1) * HD, qb * QBS : (qb + 1) * QBS],
                            cun[0:HD, :],
                            bc[0:HD, :],
                        )

                    return u

                # for the last block the caller interleaves the norm units
                # with the tail's dt0 output-projection matmuls
                return [mk_norm(0), mk_norm(1)]

            def with_norm(units, norm):
                units = list(units)
                return units[:4] + list(norm) + units[4:]

            # ---- emission schedule ----
            for grp in (
                qk_units(0, 0, wqm3, q_T),
                qk_units(0, 0, wkm3, k_T),
            ):
                for u in grp:
                    u()
            for st in range(4):
                for u in v_units(st):
                    u()
            # att(0,0) gets fillers so its ACT-serial warmup doesn't stall PE;
            # qk(0,1) drains late enough that x block 1 has landed
            n00 = attention(
                0, 0,
                qk_units(1, 0, wqm3, q_T) + qk_units(1, 0, wkm3, k_T)
                + qk_units(0, 1, wqm3, q_T),
            )
            # v(4..7) must be scheduled a block BEFORE att(0,1) reads them
            n10 = attention(
                1, 0,
                with_norm(
                    qk_units(0, 1, wkm3, k_T)
                    + v_units(4) + v_units(5) + v_units(6) + v_units(7),
                    n00,
                ),
            )
            n01 = attention(
                0, 1,
                with_norm(
                    qk_units(1, 1, wqm3, q_T) + qk_units(1, 1, wkm3, k_T), n10
                ),
            )
            n11 = attention(
                1, 1,
                with_norm(
                    qk_units(0, 2, wqm3, q_T) + qk_units(0, 2, wkm3, k_T)
                    + v_units(8) + v_units(9) + v_units(10) + v_units(11),
                    n01,
                ),
            )
            n02 = attention(
                0, 2,
                with_norm(
                    qk_units(1, 2, wqm3, q_T) + qk_units(1, 2, wkm3, k_T), n11
                ),
            )
            n12 = attention(
                1, 2,
                with_norm(
                    qk_units(0, 3, wqm3, q_T) + qk_units(0, 3, wkm3, k_T)
                    + v_units(12) + v_units(13) + v_units(14) + v_units(15)
                    + outproj_units(0) + outproj_units(1),
                    n02,
                ),
            )
            n03 = attention(
                0, 3,
                with_norm(
                    qk_units(1, 3, wqm3, q_T) + qk_units(1, 3, wkm3, k_T)
                    + outproj_units(2) + outproj_units(3)
                    + outproj_units(4) + outproj_units(5),
                    n12,
                ),
            )
            n13 = attention(
                1, 3,
                with_norm(
                    outproj_units(6) + outproj_units(7) + outproj_units(8)
                    + outproj_units(9) + outproj_units(10) + outproj_units(11),
                    n03,
                ),
                last=True,
            )

            for u in n13:
                u()
            # tail: alternate PSUM tags (ctx tag is free now) for a 4-slot
            # rotation; copies split across ACT/DVE, DMAs across HWDGE/SWDGE
            for st in range(12, 16):
                for u in outproj_units(
                    st, tag=("qk" if st % 2 == 0 else "ctx"), tail=True
                ):
                    u()
    return nc


_NC_CACHE = {}


def _get_nc() -> bass.Bass:
    if "nc" not in _NC_CACHE:
        _NC_CACHE["nc"] = build_nc()
    return _NC_CACHE["nc"]


def kernel(in_features: np.ndarray, Wqkv: np.ndarray, Wo: np.ndarray) -> np.ndarray:
    BF = ml_dtypes.bfloat16
    x32 = np.ascontiguousarray(np.asarray(in_features, dtype=np.float32))
    Wqkv = np.asarray(Wqkv, dtype=np.float32)
    Wo = np.asarray(Wo, dtype=np.float32)

    tri = np.triu(np.ones((128, 128), dtype=np.float32))  # P^T[k,q] valid iff q >= k

    def img_kpm(arr_t, k, f):
        # arr_t: [k*128, f] -> partition-major image [128, k*f]
        return np.ascontiguousarray(
            arr_t.reshape(k, 128, f).transpose(1, 0, 2).reshape(128, k * f).astype(BF)
        )

    in_maps = []
    for c in range(N_CORES):
        b, g = divmod(c, NHL)
        sl = slice(g * E, (g + 1) * E)
        in_maps.append(
            {
                "x_img": img_kpm(x32[b].T, DT, S),
                "wq_img": img_kpm(np.ascontiguousarray(Wqkv[sl, :]).T, DT, E),
                "wk_img": img_kpm(np.ascontiguousarray(Wqkv[D:][sl, :]).T, DT, E),
                "wv_img": img_kpm(np.ascontiguousarray(Wqkv[2 * D :][sl, :]).T, DT, E),
                "wo_img": img_kpm(np.ascontiguousarray(Wo[:, sl]).T, 2, D),
                "tri": tri,
                "ones4": np.ones((128, NHL), dtype=np.float32),
            }
        )

    res = run_bass_kernel_spmd(_get_nc(), in_maps, core_ids=list(range(N_CORES)))
    outs = [res.results[c]["out"].astype(np.float32) for c in range(N_CORES)]
    return np.stack(
        [outs[0] + outs[1] + outs[2] + outs[3], outs[4] + outs[5] + outs[6] + outs[7]],
        axis=0,
    )


# revision 5
# speedup vs baseline: 1.1679x; 1.0708x over previous
"""Causal MHSA Trainium2 kernel (8 NeuronCores) — v4.

Sharding: core c = 4*b + g handles batch b and head-group g (4 of 16
heads); host sums the 4 head-group partial projections per batch.

v4 (vs v3):
- All streaming tensors are bf16: x / Wq / Wk / Wv / Wo inputs arrive as
  host-prepared bf16 SBUF images (one strided DMA each, 2-4KB rows), and
  the output partial is written bf16 (host upcasts and sums). Total DMA
  drops from ~20MB to ~8MB per core, shrinking the startup window and the
  tail drain.
- q_T/k_T/ctx_T live in bf16, so the diagonal score matmuls no longer
  need >=256-col widening (bf16 runs 1 cycle/col at any width).
- Output staging is always through SBUF (bf16), never direct from PSUM.
"""

import json

import ml_dtypes
import numpy as np

import concourse.bass as bass
import concourse.mybir as mybir
import concourse.tile as tile
from concourse.bass_utils import run_bass_kernel_spmd

# ---------------------------------------------------------------------------
# Workaround: this container's walrus rejects instructions carrying more
# than one semaphore wait ("Too many sync wait commands", e.g. on the
# TileContext final drain). Split every multi-wait instruction into
# single-wait NoOps on the same engine placed immediately before it.
# ---------------------------------------------------------------------------


def _split_multiwait_bir(bir_bytes: bytes) -> bytes:
    bir = json.loads(bir_bytes)
    ctr = 0
    for fn in bir.get("functions", []):
        for bb in fn.get("blocks", []):
            out = []
            for inst in bb.get("instructions", []):
                si = inst.get("sync_info")
                waits = (si or {}).get("on_wait") or []
                if len(waits) > 1 and "engine" in inst:
                    for w in waits:
                        ctr += 1
                        out.append(
                            {
                                "debug": inst.get("debug", 0),
                                "engine": inst["engine"],
                                "ins": [],
                                "outs": [],
                                "name": f"{inst['name']}-sw{ctr}",
                                "opcode": "NoOp",
                                "sync_info": {"on_update": [], "on_wait": [w]},
                            }
                        )
                    si["on_wait"] = []
                out.append(inst)
            bb["instructions"] = out
    return json.dumps(bir).encode()


class _BassSplitWaits(bass.Bass):
    def to_json_bytes(self) -> bytes:
        return _split_multiwait_bir(super().to_json_bytes())


# ---------------------------------------------------------------------------
B = 2
S = 2048
D = 1024
HD = 64
N_CORES = 8
NHL = 4  # heads per core
E = NHL * HD  # 256
DT = D // 128  # 8
ST = S // 128  # 16
QBS = 512
NQB = S // QBS  # 4
F32 = mybir.dt.float32
F32R = mybir.dt.float32r
BF16 = mybir.dt.bfloat16
E4M3 = mybir.dt.float8e4
E5M2 = mybir.dt.float8e5
DR = mybir.MatmulPerfMode.DoubleRow
SCALE = 1.0 / np.sqrt(HD)


def build_nc() -> bass.Bass:
    nc = _BassSplitWaits()

    # host-prepared SBUF images (partition-major). QKV runs as fp8
    # DoubleRow with residual compensation: W^T x ~= W8^T x8 + W8^T dx8 +
    # dW8^T x8, where *8 are e4m3 and d* are e5m2 residuals (r = full - *8).
    # Layouts pair kt tiles for DoubleRow: x images are [p, j, t, s]
    # (kt = 2j + t), w images [p, j, t, e].
    NJ = DT // 2  # 4 kt-pairs
    x8_img = nc.dram_tensor("x8_img", [128, DT * S], E4M3, kind="ExternalInput")
    dx8_img = nc.dram_tensor("dx8_img", [128, DT * S], E5M2, kind="ExternalInput")
    w8_imgs = {
        w: nc.dram_tensor(f"{w}8_img", [128, DT * E], E4M3, kind="ExternalInput")
        for w in ("wq", "wk", "wv")
    }
    dw8_imgs = {
        w: nc.dram_tensor(f"d{w}8_img", [128, DT * E], E5M2, kind="ExternalInput")
        for w in ("wq", "wk", "wv")
    }
    wo_img = nc.dram_tensor("wo_img", [128, 2 * D], BF16, kind="ExternalInput")
    tri_in = nc.dram_tensor("tri", [128, 128], F32R, kind="ExternalInput")
    ones_in = nc.dram_tensor("ones4", [128, NHL], F32R, kind="ExternalInput")
    out = nc.dram_tensor("out", [S, D], BF16, kind="ExternalOutput")

    def dram_ap(t, base, ap):
        ref = t[0:1, 0:1]
        return bass.AP(tensor=ref.tensor, offset=base, ap=[list(a) for a in ap])

    with tile.TileContext(nc) as tc:
        with (
            tc.tile_pool(name="persist", bufs=1) as pp,
            tc.tile_pool(name="work", bufs=3) as wp,
            tc.tile_pool(name="ps", bufs=1, space="PSUM") as ps,
        ):
            # ---- mega tiles ----
            xm8 = pp.tile([128, DT * S], E4M3, name="xm8", tag="xm8")
            xm84 = xm8.rearrange("p (j t s) -> p j t s", j=NJ, t=2)
            dxm8 = pp.tile([128, DT * S], E5M2, name="dxm8", tag="dxm8")
            dxm84 = dxm8.rearrange("p (j t s) -> p j t s", j=NJ, t=2)
            w84 = {}
            dw84 = {}
            for w in ("wq", "wk", "wv"):
                t8 = pp.tile([128, DT * E], E4M3, name=f"{w}8", tag=f"{w}8")
                w84[w] = t8.rearrange("p (j t e) -> p j t e", j=NJ, t=2)
                td = pp.tile([128, DT * E], E5M2, name=f"d{w}8", tag=f"d{w}8")
                dw84[w] = td.rearrange("p (j t e) -> p j t e", j=NJ, t=2)
            wom = pp.tile([128, 2 * D], BF16, name="wom", tag="wom")
            wom3 = wom.rearrange("p (d c) -> p d c", d=2)
            tri = pp.tile([128, 128], F32R, name="tri", tag="tri")
            tri_bf = pp.tile([128, 128], BF16, name="tri_bf", tag="tri_bf")
            ones_col = pp.tile([128, NHL], F32R, name="ones_col", tag="ones_col")

            # ---- loads: strided DMAs straight off the host images, in
            # first-use order. x on Pool/SWDGE, weights on SP/HWDGE,
            # constants on the ACT queue.
            def x_dma(img, dst4, jlo, jhi, slo, shi):
                nc.gpsimd.dma_start(
                    out=dst4[:, jlo:jhi, :, slo:shi],
                    in_=dram_ap(
                        img,
                        jlo * 2 * S + slo,
                        [[DT * S, 128], [S, 2 * (jhi - jlo)], [1, shi - slo]],
                    ),
                )

            def w_dma(wdram, dst4, jlo, jhi):
                nc.sync.dma_start(
                    out=dst4[:, jlo:jhi, :, :],
                    in_=dram_ap(
                        wdram,
                        jlo * 2 * E,
                        [[DT * E, 128], [1, 2 * (jhi - jlo) * E]],
                    ),
                )

            w_dma(w8_imgs["wq"], w84["wq"], 0, 2)
            x_dma(x8_img, xm84, 0, 2, 0, QBS)
            w_dma(w8_imgs["wq"], w84["wq"], 2, 4)
            x_dma(x8_img, xm84, 2, 4, 0, QBS)
            w_dma(w8_imgs["wk"], w84["wk"], 0, 4)
            x_dma(dx8_img, dxm84, 0, 4, 0, QBS)
            w_dma(dw8_imgs["wq"], dw84["wq"], 0, 4)
            w_dma(dw8_imgs["wk"], dw84["wk"], 0, 4)
            w_dma(w8_imgs["wv"], w84["wv"], 0, 4)
            w_dma(dw8_imgs["wv"], dw84["wv"], 0, 4)
            nc.scalar.dma_start(out=tri, in_=tri_in[:, :])
            nc.scalar.dma_start(out=ones_col, in_=ones_in[:, :])
            nc.vector.tensor_copy(out=tri_bf, in_=tri)
            x_dma(x8_img, xm84, 0, 4, QBS, 2 * QBS)
            x_dma(dx8_img, dxm84, 0, 4, QBS, 2 * QBS)
            nc.sync.dma_start(
                out=wom3[:, :, :],
                in_=dram_ap(wo_img, 0, [[2 * D, 128], [1, 2 * D]]),
            )
            x_dma(x8_img, xm84, 0, 4, 2 * QBS, 3 * QBS)
            x_dma(dx8_img, dxm84, 0, 4, 2 * QBS, 3 * QBS)
            x_dma(x8_img, xm84, 0, 4, 3 * QBS, 4 * QBS)
            x_dma(dx8_img, dxm84, 0, 4, 3 * QBS, 4 * QBS)

            # ---- persistent intermediates ----
            q_T = [pp.tile([128, S], BF16, name=f"qT{p}", tag=f"qT{p}") for p in range(2)]
            k_T = [pp.tile([128, S], BF16, name=f"kT{p}", tag=f"kT{p}") for p in range(2)]
            v_aug = [
                pp.tile([128, NHL * (HD + 1)], BF16, name=f"va{st}", tag=f"va{st}")
                for st in range(ST)
            ]
            ctx_T = [pp.tile([128, S], BF16, name=f"cT{p}", tag=f"cT{p}") for p in range(2)]

            # ---- unit builders: each unit is ~2 DoubleRow matmuls or one
            # copy. Projections accumulate 3 compensated fp8 terms:
            # W8^T x8 + W8^T dx8 + dW8^T x8 (12 DR matmuls over 4 kt-pairs),
            # ordered mains-first so the residual images can arrive later.
            def qk_units(p, nb, wkey, dst):
                sl = slice(nb * QBS, (nb + 1) * QBS)
                w4, dw4 = w84[wkey], dw84[wkey]
                psl = slice(p * 128, (p + 1) * 128)
                cell = {}

                def terms(j):
                    return (
                        (w4[:, j, :, psl], xm84[:, j, :, sl]),
                        (w4[:, j, :, psl], dxm84[:, j, :, sl]),
                        (dw4[:, j, :, psl], xm84[:, j, :, sl]),
                    )

                # (term, j) emission order: mains j0..3, then residuals
                order = [(0, j) for j in range(NJ)]
                order += [(t, j) for j in range(NJ) for t in (1, 2)]

                def mk(lo, hi):
                    def u():
                        if lo == 0:
                            cell["acc"] = ps.tile(
                                [128, QBS], F32, name="acc", tag="qk", bufs=2
                            )
                        for i in range(lo, hi):
                            t, j = order[i]
                            lh, rh = terms(j)[t]
                            nc.tensor.matmul(
                                cell["acc"],
                                lhsT=lh,
                                rhs=rh,
                                start=(i == 0),
                                stop=(i == len(order) - 1),
                                perf_mode=DR,
                            )

                    return u

                units = [mk(0, 2), mk(2, 4), mk(4, 6), mk(6, 8), mk(8, 10), mk(10, 12)]

                def fin():
                    nc.vector.tensor_copy(out=dst[p][:, sl], in_=cell["acc"])

                units.append(fin)
                return units

            def v_units(st):
                stsl = slice(st * 128, (st + 1) * 128)
                cell = {}

                def terms(j):
                    return (
                        (xm84[:, j, :, stsl], w84["wv"][:, j, :, :]),
                        (dxm84[:, j, :, stsl], w84["wv"][:, j, :, :]),
                        (xm84[:, j, :, stsl], dw84["wv"][:, j, :, :]),
                    )

                order = [(0, j) for j in range(NJ)]
                order += [(t, j) for j in range(NJ) for t in (1, 2)]

                def mk(lo, hi):
                    def u():
                        if lo == 0:
                            cell["acc"] = ps.tile(
                                [128, QBS], F32, name="acc", tag="qk", bufs=2
                            )
                        for i in range(lo, hi):
                            t, j = order[i]
                            lh, rh = terms(j)[t]
                            nc.tensor.matmul(
                                cell["acc"][:, 0:E],
                                lhsT=lh,
                                rhs=rh,
                                start=(i == 0),
                                stop=(i == len(order) - 1),
                                perf_mode=DR,
                            )

                    return u

                units = [mk(0, 4), mk(4, 8), mk(8, 12)]

                def fin():
                    va = v_aug[st].rearrange("p (h c) -> p h c", h=NHL)
                    nc.vector.tensor_copy(
                        out=va[:, :, 0:HD],
                        in_=cell["acc"][:, 0:E].rearrange("p (h c) -> p h c", h=NHL),
                    )
                    nc.vector.tensor_copy(
                        out=va[:, :, HD : HD + 1],
                        in_=ones_col.rearrange("p (h c) -> p h c", c=1),
                    )

                units.append(fin)
                return units

            def outproj_units(st, tag="qk", copy_eng=None, tail=False):
                cell = {}

                def mk_mm(nb):
                    def u():
                        pso = ps.tile([128, QBS], F32, name="pso", tag=tag, bufs=2)
                        cell[nb] = pso
                        for dt_ in range(2):
                            nc.tensor.matmul(
                                pso,
                                lhsT=ctx_T[dt_][:, st * 128 : (st + 1) * 128],
                                rhs=wom3[:, dt_, nb * QBS : (nb + 1) * QBS],
                                start=(dt_ == 0),
                                stop=(dt_ == 1),
                            )

                    return u

                def mk_fin(nb, eng):
                    def u():
                        # stage via SBUF bf16 (frees the PSUM slot fast) and
                        # DMA the half right away so the tail's last DMA
                        # chain is short
                        if "osb" not in cell:
                            cell["osb"] = wp.tile(
                                [128, D], BF16, name="osb", tag="osb", bufs=4
                            )
                        if eng == "scalar":
                            nc.scalar.copy(
                                out=cell["osb"][:, nb * QBS : (nb + 1) * QBS],
                                in_=cell[nb],
                            )
                        else:
                            nc.vector.tensor_copy(
                                out=cell["osb"][:, nb * QBS : (nb + 1) * QBS],
                                in_=cell[nb],
                            )
                        # tail: odd-nb DMAs go out the SWDGE (Pool) queue so
                        # the 625ns/DMA HWDGE generation chain halves
                        dma_q = nc.gpsimd if (tail and nb == 1) else nc.sync
                        dma_q.dma_start(
                            out=out[st * 128 : (st + 1) * 128, nb * QBS : (nb + 1) * QBS],
                            in_=cell["osb"][:, nb * QBS : (nb + 1) * QBS],
                        )

                    return u

                if tail:
                    # both matmuls back-to-back (alternating PSUM tags give 4
                    # slots), staging copies split across ACT and DVE
                    return [
                        mk_mm(0),
                        mk_mm(1),
                        mk_fin(0, "scalar"),
                        mk_fin(1, "vector"),
                    ]
                return [mk_mm(0), mk_fin(0, copy_eng), mk_mm(1), mk_fin(1, copy_eng)]

            # ---- attention block with deferred normalization ----
            def attention(p, qb, fillers=(), last=False):
                fillers = list(fillers)
                n_kt = 4 * qb + 4
                ctxs = [
                    ps.tile([128, QBS], F32, name=f"ctx{h}", tag="ctx", bufs=2)
                    for h in range(2)
                ]
                pts = {}
                for kt in range(n_kt + 1):
                    if kt < n_kt:
                        o = 0 if kt < 4 * qb else (kt - 4 * qb) * 128
                        s_ps = ps.tile([128, 2 * QBS], F32, name="s_ps", tag="s", bufs=2)
                        for hl in range(2):
                            nc.tensor.matmul(
                                s_ps[:, hl * QBS + o : (hl + 1) * QBS],
                                lhsT=k_T[p][
                                    hl * HD : (hl + 1) * HD, kt * 128 : (kt + 1) * 128
                                ],
                                rhs=q_T[p][
                                    hl * HD : (hl + 1) * HD,
                                    qb * QBS + o : (qb + 1) * QBS,
                                ],
                                start=True,
                                stop=True,
                            )
                        pt = wp.tile([128, 2 * QBS], BF16, name="pt", tag="pt", bufs=4)
                        sv = s_ps.rearrange("p (h q) -> p h q", h=2)
                        pv = pt.rearrange("p (h q) -> p h q", h=2)
                        nc.scalar.activation(
                            out=pv[:, :, o:QBS],
                            in_=sv[:, :, o:QBS],
                            func=mybir.ActivationFunctionType.Exp,
                            scale=float(SCALE),
                        )
                        if kt >= 4 * qb:
                            for hl in range(2):
                                blk = pt[:, hl * QBS + o : hl * QBS + o + 128]
                                nc.vector.tensor_mul(blk, blk, tri_bf)
                        pts[kt] = (pt, o)
                    # fillers BEFORE ctx(kt-1): PE executes in order, so the
                    # (independent) fillers run while exp(kt-1) finishes; the
                    # ctx matmul then starts without exposing the ACT latency.
                    # Pops adapt so the filler list drains evenly across the
                    # block instead of leaving a burst stuck behind the last
                    # (dependency-carrying) ctx matmul.
                    iters_left = n_kt + 1 - kt
                    if last:
                        npop = 1
                    else:
                        npop = max(2, -(-len(fillers) // iters_left))
                    for _ in range(npop):
                        if fillers:
                            fillers.pop(0)()
                    if kt > 0:
                        pt, o = pts.pop(kt - 1)
                        for hl in range(2):
                            nc.tensor.matmul(
                                ctxs[hl][0 : HD + 1, o:QBS],
                                lhsT=v_aug[kt - 1][
                                    :, (2 * p + hl) * (HD + 1) : (2 * p + hl + 1) * (HD + 1)
                                ],
                                rhs=pt[:, hl * QBS + o : (hl + 1) * QBS],
                                start=(kt - 1 == 0),
                                stop=(kt - 1 == n_kt - 1),
                                skip_group_check=True,
                            )
                # stage unnormalized ctx through SBUF + reciprocal on the
                # denominator row; the broadcast+multiply is deferred
                cuns = []
                for hl in range(2):
                    cun = wp.tile([HD + 1, QBS], F32R, name="cun", tag="cun", bufs=4)
                    if last:
                        # keep the tail's DVE budget for recips/norm muls and
                        # staging copies; ACT has no exps left here
                        nc.scalar.copy(out=cun, in_=ctxs[hl][0 : HD + 1, :])
                    else:
                        nc.vector.tensor_copy(out=cun, in_=ctxs[hl][0 : HD + 1, :])
                    # in-place reciprocal at partition 64 (equal in/out base —
                    # a DVE input at partition 64 with output at partition 0
                    # reads wrong data on HW)
                    with nc.allow_low_precision(reason="f32r is bitwise f32"):
                        nc.vector.reciprocal(
                            out=cun[HD : HD + 1, :], in_=cun[HD : HD + 1, :]
                        )
                    cuns.append(cun)
                while fillers:
                    fillers.pop(0)()

                def mk_norm(hl):
                    cun = cuns[hl]

                    def u():
                        # broadcast recip row across 64 partitions with a
                        # 1-row matmul: ones(1,64)^T @ recip(1,QBS)
                        bc = ps.tile([128, QBS], F32, name="bc", tag="qk", bufs=2)
                        # tri row 64 cols 64:128 is all-ones at partition 64,
                        # matching the recip row's base partition
                        nc.tensor.matmul(
                            bc[0:HD, :],
                            lhsT=tri[HD : HD + 1, HD : 2 * HD],
                            rhs=cun[HD : HD + 1, :],
                            start=True,
                            stop=True,
                        )
                        nc.vector.tensor_mul(
                            ctx_T[p][hl * HD : (hl + 1) * HD, qb * QBS : (qb + 1) * QBS],
                            cun[0:HD, :],
                            bc[0:HD, :],
                        )

                    return u

                # for the last block the caller interleaves the norm units
                # with the tail's dt0 output-projection matmuls
                return [mk_norm(0), mk_norm(1)]

            def with_norm(units, norm):
                units = list(units)
                return units[:4] + list(norm) + units[4:]

            # ---- emission schedule ----
            # startup: mains (w8+x8 only) before residuals so PE consumes in
            # DMA-arrival order; two accs alive at a time (qk tag bufs=2)
            qg = qk_units(0, 0, "wq", q_T)
            kg = qk_units(0, 0, "wk", k_T)
            for u in (qg[0], qg[1], kg[0], kg[1]):
                u()
            for u in qg[2:]:
                u()
            for u in kg[2:]:
                u()
            vg = [v_units(st) for st in range(4)]
            vg[0][0]()
            vg[1][0]()
            for u in vg[0][1:]:
                u()
            for u in vg[1][1:]:
                u()
            vg[2][0]()
            vg[3][0]()
            for u in vg[2][1:]:
                u()
            for u in vg[3][1:]:
                u()
            # att(0,0) gets fillers so its ACT-serial warmup doesn't stall PE;
            # qk(0,1) drains late enough that x block 1 has landed
            n00 = attention(
                0, 0,
                qk_units(1, 0, "wq", q_T) + qk_units(1, 0, "wk", k_T)
                + qk_units(0, 1, "wq", q_T),
            )
            # v(4..7) must be scheduled a block BEFORE att(0,1) reads them
            n10 = attention(
                1, 0,
                with_norm(
                    qk_units(0, 1, "wk", k_T)
                    + v_units(4) + v_units(5) + v_units(6) + v_units(7),
                    n00,
                ),
            )
            n01 = attention(
                0, 1,
                with_norm(
                    qk_units(1, 1, "wq", q_T) + qk_units(1, 1, "wk", k_T), n10
                ),
            )
            n11 = attention(
                1, 1,
                with_norm(
                    qk_units(0, 2, "wq", q_T) + qk_units(0, 2, "wk", k_T)
                    + v_units(8) + v_units(9) + v_units(10) + v_units(11),
                    n01,
                ),
            )
            n02 = attention(
                0, 2,
                with_norm(
                    qk_units(1, 2, "wq", q_T) + qk_units(1, 2, "wk", k_T), n11
                ),
            )
            n12 = attention(
                1, 2,
                with_norm(
                    qk_units(0, 3, "wq", q_T) + qk_units(0, 3, "wk", k_T)
                    + v_units(12) + v_units(13) + v_units(14) + v_units(15)
                    + outproj_units(0) + outproj_units(1),
                    n02,
                ),
            )
            n03 = attention(
                0, 3,
                with_norm(
                    qk_units(1, 3, "wq", q_T) + qk_units(1, 3, "wk", k_T)
                    + outproj_units(2) + outproj_units(3)
                    + outproj_units(4) + outproj_units(5),
                    n12,
                ),
            )
            n13 = attention(
                1, 3,
                with_norm(
                    outproj_units(6) + outproj_units(7) + outproj_units(8)
                    + outproj_units(9) + outproj_units(10) + outproj_units(11),
                    n03,
                ),
                last=True,
            )

            for u in n13:
                u()
            # tail: alternate PSUM tags (ctx tag is free now) for a 4-slot
            # rotation; copies split across ACT/DVE, DMAs across HWDGE/SWDGE
            for st in range(12, 16):
                for u in outproj_units(
                    st, tag=("qk" if st % 2 == 0 else "ctx"), tail=True
                ):
                    u()
    return nc


_NC_CACHE = {}


def _get_nc() -> bass.Bass:
    if "nc" not in _NC_CACHE:
        _NC_CACHE["nc"] = build_nc()
    return _NC_CACHE["nc"]


def kernel(in_features: np.ndarray, Wqkv: np.ndarray, Wo: np.ndarray) -> np.ndarray:
    BF = ml_dtypes.bfloat16
    E4 = ml_dtypes.float8_e4m3
    E5 = ml_dtypes.float8_e5m2
    NJ = DT // 2
    x32 = np.ascontiguousarray(np.asarray(in_features, dtype=np.float32))
    Wqkv = np.asarray(Wqkv, dtype=np.float32)
    Wo = np.asarray(Wo, dtype=np.float32)

    tri = np.triu(np.ones((128, 128), dtype=np.float32))  # P^T[k,q] valid iff q >= k

    def img_kpm(arr_t, k, f):
        # arr_t: [k*128, f] -> partition-major image [128, k*f]
        return np.ascontiguousarray(
            arr_t.reshape(k, 128, f).transpose(1, 0, 2).reshape(128, k * f).astype(BF)
        )

    def img_pair(a, f):
        # [DT*128, f] -> kt-paired partition-major image [128, NJ*2*f]
        return np.ascontiguousarray(
            a.reshape(NJ, 2, 128, f).transpose(2, 0, 1, 3).reshape(128, DT * f)
        )

    def split8(arr_t, f):
        # fp8 main (e4m3) + residual (e5m2) pair-layout images
        a8 = arr_t.astype(E4)
        d8 = (arr_t - a8.astype(np.float32)).astype(E5)
        return img_pair(a8, f), img_pair(d8, f)

    in_maps = []
    for c in range(N_CORES):
        b, g = divmod(c, NHL)
        sl = slice(g * E, (g + 1) * E)
        x8, dx8 = split8(x32[b].T, S)
        wq8, dwq8 = split8(np.ascontiguousarray(Wqkv[sl, :]).T, E)
        wk8, dwk8 = split8(np.ascontiguousarray(Wqkv[D:][sl, :]).T, E)
        wv8, dwv8 = split8(np.ascontiguousarray(Wqkv[2 * D :][sl, :]).T, E)
        in_maps.append(
            {
                "x8_img": x8,
                "dx8_img": dx8,
                "wq8_img": wq8,
                "dwq8_img": dwq8,
                "wk8_img": wk8,
                "dwk8_img": dwk8,
                "wv8_img": wv8,
                "dwv8_img": dwv8,
                "wo_img": img_kpm(np.ascontiguousarray(Wo[:, sl]).T, 2, D),
                "tri": tri,
                "ones4": np.ones((128, NHL), dtype=np.float32),
            }
        )

    res = run_bass_kernel_spmd(_get_nc(), in_maps, core_ids=list(range(N_CORES)))
    outs = [res.results[c]["out"].astype(np.float32) for c in range(N_CORES)]
    return np.stack(
        [outs[0] + outs[1] + outs[2] + outs[3], outs[4] + outs[5] + outs[6] + outs[7]],
        axis=0,
    )


# revision 6
# speedup vs baseline: 1.1854x; 1.0150x over previous
"""Causal MHSA Trainium2 kernel (8 NeuronCores) — v4.

Sharding: core c = 4*b + g handles batch b and head-group g (4 of 16
heads); host sums the 4 head-group partial projections per batch.

v4 (vs v3):
- All streaming tensors are bf16: x / Wq / Wk / Wv / Wo inputs arrive as
  host-prepared bf16 SBUF images (one strided DMA each, 2-4KB rows), and
  the output partial is written bf16 (host upcasts and sums). Total DMA
  drops from ~20MB to ~8MB per core, shrinking the startup window and the
  tail drain.
- q_T/k_T/ctx_T live in bf16, so the diagonal score matmuls no longer
  need >=256-col widening (bf16 runs 1 cycle/col at any width).
- Output staging is always through SBUF (bf16), never direct from PSUM.
"""

import json

import ml_dtypes
import numpy as np

import concourse.bass as bass
import concourse.mybir as mybir
import concourse.tile as tile
from concourse.bass_utils import run_bass_kernel_spmd

# ---------------------------------------------------------------------------
# Workaround: this container's walrus rejects instructions carrying more
# than one semaphore wait ("Too many sync wait commands", e.g. on the
# TileContext final drain). Split every multi-wait instruction into
# single-wait NoOps on the same engine placed immediately before it.
# ---------------------------------------------------------------------------


def _split_multiwait_bir(bir_bytes: bytes) -> bytes:
    bir = json.loads(bir_bytes)
    ctr = 0
    for fn in bir.get("functions", []):
        for bb in fn.get("blocks", []):
            out = []
            for inst in bb.get("instructions", []):
                si = inst.get("sync_info")
                waits = (si or {}).get("on_wait") or []
                if len(waits) > 1 and "engine" in inst:
                    for w in waits:
                        ctr += 1
                        out.append(
                            {
                                "debug": inst.get("debug", 0),
                                "engine": inst["engine"],
                                "ins": [],
                                "outs": [],
                                "name": f"{inst['name']}-sw{ctr}",
                                "opcode": "NoOp",
                                "sync_info": {"on_update": [], "on_wait": [w]},
                            }
                        )
                    si["on_wait"] = []
                out.append(inst)
            bb["instructions"] = out
    return json.dumps(bir).encode()


class _BassSplitWaits(bass.Bass):
    def to_json_bytes(self) -> bytes:
        return _split_multiwait_bir(super().to_json_bytes())


# ---------------------------------------------------------------------------
B = 2
S = 2048
D = 1024
HD = 64
N_CORES = 8
NHL = 4  # heads per core
E = NHL * HD  # 256
DT = D // 128  # 8
ST = S // 128  # 16
QBS = 512
NQB = S // QBS  # 4
F32 = mybir.dt.float32
F32R = mybir.dt.float32r
BF16 = mybir.dt.bfloat16
E4M3 = mybir.dt.float8e4
E5M2 = mybir.dt.float8e5
DR = mybir.MatmulPerfMode.DoubleRow
SCALE = 1.0 / np.sqrt(HD)


def build_nc() -> bass.Bass:
    nc = _BassSplitWaits()

    # host-prepared SBUF images (partition-major). QKV runs as fp8
    # DoubleRow with residual compensation: W^T x ~= W8^T x8 + W8^T dx8 +
    # dW8^T x8, where *8 are e4m3 and d* are e5m2 residuals (r = full - *8).
    # Layouts pair kt tiles for DoubleRow: x images are [p, j, t, s]
    # (kt = 2j + t), w images [p, j, t, e].
    NJ = DT // 2  # 4 kt-pairs
    x8_img = nc.dram_tensor("x8_img", [128, DT * S], E4M3, kind="ExternalInput")
    dx8_img = nc.dram_tensor("dx8_img", [128, DT * S], E5M2, kind="ExternalInput")
    w8_imgs = {
        w: nc.dram_tensor(f"{w}8_img", [128, DT * E], E4M3, kind="ExternalInput")
        for w in ("wq", "wk", "wv")
    }
    dw8_imgs = {
        w: nc.dram_tensor(f"d{w}8_img", [128, DT * E], E5M2, kind="ExternalInput")
        for w in ("wq", "wk", "wv")
    }
    wo_img = nc.dram_tensor("wo_img", [128, 2 * D], BF16, kind="ExternalInput")
    tri_in = nc.dram_tensor("tri", [128, 128], F32R, kind="ExternalInput")
    ones_in = nc.dram_tensor("ones4", [128, NHL], F32R, kind="ExternalInput")
    out = nc.dram_tensor("out", [S, D], BF16, kind="ExternalOutput")

    def dram_ap(t, base, ap):
        ref = t[0:1, 0:1]
        return bass.AP(tensor=ref.tensor, offset=base, ap=[list(a) for a in ap])

    with tile.TileContext(nc) as tc:
        with (
            tc.tile_pool(name="persist", bufs=1) as pp,
            tc.tile_pool(name="work", bufs=3) as wp,
            tc.tile_pool(name="ps", bufs=1, space="PSUM") as ps,
        ):
            # ---- mega tiles ----
            xm8 = pp.tile([128, DT * S], E4M3, name="xm8", tag="xm8")
            xm84 = xm8.rearrange("p (j t s) -> p j t s", j=NJ, t=2)
            dxm8 = pp.tile([128, DT * S], E5M2, name="dxm8", tag="dxm8")
            dxm84 = dxm8.rearrange("p (j t s) -> p j t s", j=NJ, t=2)
            w84 = {}
            dw84 = {}
            for w in ("wq", "wk", "wv"):
                t8 = pp.tile([128, DT * E], E4M3, name=f"{w}8", tag=f"{w}8")
                w84[w] = t8.rearrange("p (j t e) -> p j t e", j=NJ, t=2)
                td = pp.tile([128, DT * E], E5M2, name=f"d{w}8", tag=f"d{w}8")
                dw84[w] = td.rearrange("p (j t e) -> p j t e", j=NJ, t=2)
            wom = pp.tile([128, 2 * D], BF16, name="wom", tag="wom")
            wom3 = wom.rearrange("p (d c) -> p d c", d=2)
            tri = pp.tile([128, 128], F32R, name="tri", tag="tri")
            tri_bf = pp.tile([128, 128], BF16, name="tri_bf", tag="tri_bf")
            ones_col = pp.tile([128, NHL], F32R, name="ones_col", tag="ones_col")

            # ---- loads: strided DMAs straight off the host images, in
            # first-use order. x on Pool/SWDGE, weights on SP/HWDGE,
            # constants on the ACT queue.
            def x_dma(img, dst4, jlo, jhi, slo, shi):
                nc.gpsimd.dma_start(
                    out=dst4[:, jlo:jhi, :, slo:shi],
                    in_=dram_ap(
                        img,
                        jlo * 2 * S + slo,
                        [[DT * S, 128], [S, 2 * (jhi - jlo)], [1, shi - slo]],
                    ),
                )

            def w_dma(wdram, dst4, jlo, jhi):
                nc.sync.dma_start(
                    out=dst4[:, jlo:jhi, :, :],
                    in_=dram_ap(
                        wdram,
                        jlo * 2 * E,
                        [[DT * E, 128], [1, 2 * (jhi - jlo) * E]],
                    ),
                )

            w_dma(w8_imgs["wq"], w84["wq"], 0, 2)
            x_dma(x8_img, xm84, 0, 2, 0, QBS)
            w_dma(w8_imgs["wq"], w84["wq"], 2, 4)
            x_dma(x8_img, xm84, 2, 4, 0, QBS)
            w_dma(w8_imgs["wk"], w84["wk"], 0, 4)
            x_dma(dx8_img, dxm84, 0, 2, 0, QBS)
            w_dma(dw8_imgs["wq"], dw84["wq"], 0, 4)
            x_dma(dx8_img, dxm84, 2, 4, 0, QBS)
            w_dma(dw8_imgs["wk"], dw84["wk"], 0, 4)
            w_dma(w8_imgs["wv"], w84["wv"], 0, 4)
            w_dma(dw8_imgs["wv"], dw84["wv"], 0, 4)
            nc.scalar.dma_start(out=tri, in_=tri_in[:, :])
            nc.scalar.dma_start(out=ones_col, in_=ones_in[:, :])
            nc.vector.tensor_copy(out=tri_bf, in_=tri)
            x_dma(x8_img, xm84, 0, 4, QBS, 2 * QBS)
            x_dma(dx8_img, dxm84, 0, 4, QBS, 2 * QBS)
            nc.sync.dma_start(
                out=wom3[:, :, :],
                in_=dram_ap(wo_img, 0, [[2 * D, 128], [1, 2 * D]]),
            )
            x_dma(x8_img, xm84, 0, 4, 2 * QBS, 3 * QBS)
            x_dma(dx8_img, dxm84, 0, 4, 2 * QBS, 3 * QBS)
            x_dma(x8_img, xm84, 0, 4, 3 * QBS, 4 * QBS)
            x_dma(dx8_img, dxm84, 0, 4, 3 * QBS, 4 * QBS)

            # ---- persistent intermediates ----
            q_T = [pp.tile([128, S], BF16, name=f"qT{p}", tag=f"qT{p}") for p in range(2)]
            k_T = [pp.tile([128, S], BF16, name=f"kT{p}", tag=f"kT{p}") for p in range(2)]
            v_aug = [
                pp.tile([128, NHL * (HD + 1)], BF16, name=f"va{st}", tag=f"va{st}")
                for st in range(ST)
            ]
            ctx_T = [pp.tile([128, S], BF16, name=f"cT{p}", tag=f"cT{p}") for p in range(2)]

            # ---- unit builders: each unit is ~2 DoubleRow matmuls or one
            # copy. Projections accumulate 3 compensated fp8 terms:
            # W8^T x8 + W8^T dx8 + dW8^T x8 (12 DR matmuls over 4 kt-pairs),
            # ordered mains-first so the residual images can arrive later.
            def qk_units(p, nb, wkey, dst):
                sl = slice(nb * QBS, (nb + 1) * QBS)
                w4, dw4 = w84[wkey], dw84[wkey]
                psl = slice(p * 128, (p + 1) * 128)
                cell = {}

                def terms(j):
                    return (
                        (w4[:, j, :, psl], xm84[:, j, :, sl]),
                        (w4[:, j, :, psl], dxm84[:, j, :, sl]),
                        (dw4[:, j, :, psl], xm84[:, j, :, sl]),
                    )

                # (term, j) emission order: mains j0..3, then residuals
                order = [(0, j) for j in range(NJ)]
                order += [(t, j) for j in range(NJ) for t in (1, 2)]

                def mk(lo, hi):
                    def u():
                        if lo == 0:
                            cell["acc"] = ps.tile(
                                [128, QBS], F32, name="acc", tag="qk", bufs=2
                            )
                        for i in range(lo, hi):
                            t, j = order[i]
                            lh, rh = terms(j)[t]
                            nc.tensor.matmul(
                                cell["acc"],
                                lhsT=lh,
                                rhs=rh,
                                start=(i == 0),
                                stop=(i == len(order) - 1),
                                perf_mode=DR,
                            )

                    return u

                units = [mk(0, 2), mk(2, 4), mk(4, 6), mk(6, 8), mk(8, 10), mk(10, 12)]

                def fin():
                    nc.vector.tensor_copy(out=dst[p][:, sl], in_=cell["acc"])

                units.append(fin)
                return units

            def v_units(st):
                stsl = slice(st * 128, (st + 1) * 128)
                cell = {}

                def terms(j):
                    return (
                        (xm84[:, j, :, stsl], w84["wv"][:, j, :, :]),
                        (dxm84[:, j, :, stsl], w84["wv"][:, j, :, :]),
                        (xm84[:, j, :, stsl], dw84["wv"][:, j, :, :]),
                    )

                order = [(0, j) for j in range(NJ)]
                order += [(t, j) for j in range(NJ) for t in (1, 2)]

                def mk(lo, hi):
                    def u():
                        if lo == 0:
                            cell["acc"] = ps.tile(
                                [128, QBS], F32, name="acc", tag="qk", bufs=2
                            )
                        for i in range(lo, hi):
                            t, j = order[i]
                            lh, rh = terms(j)[t]
                            nc.tensor.matmul(
                                cell["acc"][:, 0:E],
                                lhsT=lh,
                                rhs=rh,
                                start=(i == 0),
                                stop=(i == len(order) - 1),
                                perf_mode=DR,
                            )

                    return u

                units = [mk(0, 4), mk(4, 8), mk(8, 12)]

                def fin():
                    va = v_aug[st].rearrange("p (h c) -> p h c", h=NHL)
                    nc.vector.tensor_copy(
                        out=va[:, :, 0:HD],
                        in_=cell["acc"][:, 0:E].rearrange("p (h c) -> p h c", h=NHL),
                    )
                    nc.vector.tensor_copy(
                        out=va[:, :, HD : HD + 1],
                        in_=ones_col.rearrange("p (h c) -> p h c", c=1),
                    )

                units.append(fin)
                return units

            eng_mode = {"drain": False}

            def outproj_units(st, tag="qk", copy_eng=None, tail=False):
                cell = {}

                def mk_mm(nb):
                    def u():
                        pso = ps.tile([128, QBS], F32, name="pso", tag=tag, bufs=2)
                        cell[nb] = pso
                        for dt_ in range(2):
                            nc.tensor.matmul(
                                pso,
                                lhsT=ctx_T[dt_][:, st * 128 : (st + 1) * 128],
                                rhs=wom3[:, dt_, nb * QBS : (nb + 1) * QBS],
                                start=(dt_ == 0),
                                stop=(dt_ == 1),
                            )

                    return u

                def mk_fin(nb, eng):
                    def u():
                        # stage via SBUF bf16 (frees the PSUM slot fast) and
                        # DMA the half right away so the tail's last DMA
                        # chain is short
                        if "osb" not in cell:
                            cell["osb"] = wp.tile(
                                [128, D], BF16, name="osb", tag="osb", bufs=4
                            )
                        if eng == "scalar":
                            nc.scalar.copy(
                                out=cell["osb"][:, nb * QBS : (nb + 1) * QBS],
                                in_=cell[nb],
                            )
                        else:
                            nc.vector.tensor_copy(
                                out=cell["osb"][:, nb * QBS : (nb + 1) * QBS],
                                in_=cell[nb],
                            )
                        # tail: odd-nb DMAs go out the SWDGE (Pool) queue so
                        # the 625ns/DMA HWDGE generation chain halves
                        dma_q = nc.gpsimd if (tail and nb == 1) else nc.sync
                        dma_q.dma_start(
                            out=out[st * 128 : (st + 1) * 128, nb * QBS : (nb + 1) * QBS],
                            in_=cell["osb"][:, nb * QBS : (nb + 1) * QBS],
                        )

                    return u

                if tail:
                    # both matmuls back-to-back (alternating PSUM tags give 4
                    # slots), staging copies split across ACT and DVE, and a
                    # single full-row DMA per st (fewer DGE generations on
                    # the critical tail)
                    def copy_only(nb, eng):
                        def u():
                            if "osb" not in cell:
                                cell["osb"] = wp.tile(
                                    [128, D], BF16, name="osb", tag="osb", bufs=4
                                )
                            dst = cell["osb"][:, nb * QBS : (nb + 1) * QBS]
                            if eng == "scalar":
                                nc.scalar.copy(out=dst, in_=cell[nb])
                            else:
                                nc.vector.tensor_copy(out=dst, in_=cell[nb])

                        return u

                    def full_dma():
                        def u():
                            (nc.gpsimd if st % 2 == 0 else nc.sync).dma_start(
                                out=out[st * 128 : (st + 1) * 128, :],
                                in_=cell["osb"],
                            )

                        return u

                    return [
                        mk_mm(0),
                        mk_mm(1),
                        copy_only(0, "scalar"),
                        copy_only(1, "vector"),
                        full_dma(),
                    ]
                return [mk_mm(0), mk_fin(0, copy_eng), mk_mm(1), mk_fin(1, copy_eng)]

            # ---- attention block with deferred normalization ----
            def attention(p, qb, fillers=(), last=False):
                fillers = list(fillers)
                n_kt = 4 * qb + 4
                ctxs = [
                    ps.tile([128, QBS], F32, name=f"ctx{h}", tag="ctx", bufs=2)
                    for h in range(2)
                ]
                pts = {}
                for kt in range(n_kt + 1):
                    if kt < n_kt:
                        o = 0 if kt < 4 * qb else (kt - 4 * qb) * 128
                        s_ps = ps.tile([128, 2 * QBS], F32, name="s_ps", tag="s", bufs=2)
                        for hl in range(2):
                            nc.tensor.matmul(
                                s_ps[:, hl * QBS + o : (hl + 1) * QBS],
                                lhsT=k_T[p][
                                    hl * HD : (hl + 1) * HD, kt * 128 : (kt + 1) * 128
                                ],
                                rhs=q_T[p][
                                    hl * HD : (hl + 1) * HD,
                                    qb * QBS + o : (qb + 1) * QBS,
                                ],
                                start=True,
                                stop=True,
                            )
                        pt = wp.tile([128, 2 * QBS], BF16, name="pt", tag="pt", bufs=4)
                        sv = s_ps.rearrange("p (h q) -> p h q", h=2)
                        pv = pt.rearrange("p (h q) -> p h q", h=2)
                        nc.scalar.activation(
                            out=pv[:, :, o:QBS],
                            in_=sv[:, :, o:QBS],
                            func=mybir.ActivationFunctionType.Exp,
                            scale=float(SCALE),
                        )
                        if kt >= 4 * qb:
                            for hl in range(2):
                                blk = pt[:, hl * QBS + o : hl * QBS + o + 128]
                                nc.vector.tensor_mul(blk, blk, tri_bf)
                        pts[kt] = (pt, o)
                    # fillers BEFORE ctx(kt-1): PE executes in order, so the
                    # (independent) fillers run while exp(kt-1) finishes; the
                    # ctx matmul then starts without exposing the ACT latency.
                    # Pops adapt so the filler list drains evenly across the
                    # block instead of leaving a burst stuck behind the last
                    # (dependency-carrying) ctx matmul.
                    iters_left = n_kt + 1 - kt
                    if last:
                        npop = 1
                    else:
                        npop = max(2, -(-len(fillers) // iters_left))
                    for _ in range(npop):
                        if fillers:
                            fillers.pop(0)()
                    if kt > 0:
                        pt, o = pts.pop(kt - 1)
                        for hl in range(2):
                            nc.tensor.matmul(
                                ctxs[hl][0 : HD + 1, o:QBS],
                                lhsT=v_aug[kt - 1][
                                    :, (2 * p + hl) * (HD + 1) : (2 * p + hl + 1) * (HD + 1)
                                ],
                                rhs=pt[:, hl * QBS + o : (hl + 1) * QBS],
                                start=(kt - 1 == 0),
                                stop=(kt - 1 == n_kt - 1),
                                skip_group_check=True,
                            )
                # stage unnormalized ctx through SBUF + reciprocal on the
                # denominator row; the broadcast+multiply is deferred
                cuns = []
                for hl in range(2):
                    cun = wp.tile([HD + 1, QBS], F32R, name="cun", tag="cun", bufs=4)
                    if last:
                        # keep the tail's DVE budget for recips/norm muls and
                        # staging copies; ACT has no exps left here
                        nc.scalar.copy(out=cun, in_=ctxs[hl][0 : HD + 1, :])
                    else:
                        nc.vector.tensor_copy(out=cun, in_=ctxs[hl][0 : HD + 1, :])
                    # in-place reciprocal at partition 64 (equal in/out base —
                    # a DVE input at partition 64 with output at partition 0
                    # reads wrong data on HW)
                    with nc.allow_low_precision(reason="f32r is bitwise f32"):
                        nc.vector.reciprocal(
                            out=cun[HD : HD + 1, :], in_=cun[HD : HD + 1, :]
                        )
                    cuns.append(cun)
                if last:
                    # post-loop drain copies go to ACT so DVE stays clear
                    # for the norm chain
                    eng_mode["drain"] = True
                while fillers:
                    fillers.pop(0)()

                def mk_norm(hl):
                    cun = cuns[hl]

                    def u():
                        # broadcast recip row across 64 partitions with a
                        # 1-row matmul: ones(1,64)^T @ recip(1,QBS)
                        bc = ps.tile(
                            [128, QBS], F32, name="bc",
                            tag=("s" if last else "qk"), bufs=2,
                        )
                        # tri row 64 cols 64:128 is all-ones at partition 64,
                        # matching the recip row's base partition
                        nc.tensor.matmul(
                            bc[0:HD, :],
                            lhsT=tri[HD : HD + 1, HD : 2 * HD],
                            rhs=cun[HD : HD + 1, :],
                            start=True,
                            stop=True,
                        )
                        nc.vector.tensor_mul(
                            ctx_T[p][hl * HD : (hl + 1) * HD, qb * QBS : (qb + 1) * QBS],
                            cun[0:HD, :],
                            bc[0:HD, :],
                        )

                    return u

                # for the last block the caller interleaves the norm units
                # with the tail's dt0 output-projection matmuls
                return [mk_norm(0), mk_norm(1)]

            def with_norm(units, norm):
                units = list(units)
                return units[:4] + list(norm) + units[4:]

            # ---- emission schedule ----
            # startup: mains (w8+x8 only) before residuals so PE consumes in
            # DMA-arrival order; two accs alive at a time (qk tag bufs=2)
            qg = qk_units(0, 0, "wq", q_T)
            kg = qk_units(0, 0, "wk", k_T)
            for u in (qg[0], qg[1], kg[0], kg[1]):
                u()
            for u in qg[2:]:
                u()
            for u in kg[2:]:
                u()
            vg = [v_units(st) for st in range(4)]
            vg[0][0]()
            vg[1][0]()
            for u in vg[0][1:]:
                u()
            for u in vg[1][1:]:
                u()
            vg[2][0]()
            vg[3][0]()
            for u in vg[2][1:]:
                u()
            for u in vg[3][1:]:
                u()
            # att(0,0) gets fillers so its ACT-serial warmup doesn't stall PE;
            # qk(0,1) drains late enough that x block 1 has landed
            n00 = attention(
                0, 0,
                qk_units(1, 0, "wq", q_T) + qk_units(1, 0, "wk", k_T)
                + qk_units(0, 1, "wq", q_T),
            )
            # v(4..7) must be scheduled a block BEFORE att(0,1) reads them
            n10 = attention(
                1, 0,
                with_norm(
                    qk_units(0, 1, "wk", k_T)
                    + v_units(4) + v_units(5) + v_units(6) + v_units(7),
                    n00,
                ),
            )
            n01 = attention(
                0, 1,
                with_norm(
                    qk_units(1, 1, "wq", q_T) + qk_units(1, 1, "wk", k_T), n10
                ),
            )
            n11 = attention(
                1, 1,
                with_norm(
                    qk_units(0, 2, "wq", q_T) + qk_units(0, 2, "wk", k_T)
                    + v_units(8) + v_units(9) + v_units(10) + v_units(11),
                    n01,
                ),
            )
            n02 = attention(
                0, 2,
                with_norm(
                    qk_units(1, 2, "wq", q_T) + qk_units(1, 2, "wk", k_T), n11
                ),
            )
            n12 = attention(
                1, 2,
                with_norm(
                    qk_units(0, 3, "wq", q_T) + qk_units(0, 3, "wk", k_T)
                    + v_units(12) + v_units(13) + v_units(14) + v_units(15)
                    + outproj_units(0) + outproj_units(1),
                    n02,
                ),
            )
            n03 = attention(
                0, 3,
                with_norm(
                    qk_units(1, 3, "wq", q_T) + qk_units(1, 3, "wk", k_T)
                    + outproj_units(2) + outproj_units(3)
                    + outproj_units(4) + outproj_units(5),
                    n12,
                ),
            )
            n13 = attention(
                1, 3,
                with_norm(
                    outproj_units(6) + outproj_units(7) + outproj_units(8)
                    + outproj_units(9) + outproj_units(10) + outproj_units(11),
                    n03,
                ),
                last=True,
            )

            for u in n13:
                u()
            # tail: alternate PSUM tags (ctx tag is free now) for a 4-slot
            # rotation; copies split across ACT/DVE, DMAs across HWDGE/SWDGE
            for st in range(12, 16):
                for u in outproj_units(
                    st, tag=("qk" if st % 2 == 0 else "ctx"), tail=True
                ):
                    u()
    return nc


_NC_CACHE = {}


def _get_nc() -> bass.Bass:
    if "nc" not in _NC_CACHE:
        _NC_CACHE["nc"] = build_nc()
    return _NC_CACHE["nc"]


def kernel(in_features: np.ndarray, Wqkv: np.ndarray, Wo: np.ndarray) -> np.ndarray:
    BF = ml_dtypes.bfloat16
    E4 = ml_dtypes.float8_e4m3
    E5 = ml_dtypes.float8_e5m2
    NJ = DT // 2
    x32 = np.ascontiguousarray(np.asarray(in_features, dtype=np.float32))
    Wqkv = np.asarray(Wqkv, dtype=np.float32)
    Wo = np.asarray(Wo, dtype=np.float32)

    tri = np.triu(np.ones((128, 128), dtype=np.float32))  # P^T[k,q] valid iff q >= k

    def img_kpm(arr_t, k, f):
        # arr_t: [k*128, f] -> partition-major image [128, k*f]
        return np.ascontiguousarray(
            arr_t.reshape(k, 128, f).transpose(1, 0, 2).reshape(128, k * f).astype(BF)
        )

    def img_pair(a, f):
        # [DT*128, f] -> kt-paired partition-major image [128, NJ*2*f]
        return np.ascontiguousarray(
            a.reshape(NJ, 2, 128, f).transpose(2, 0, 1, 3).reshape(128, DT * f)
        )

    def split8(arr_t, f):
        # fp8 main (e4m3) + residual (e5m2) pair-layout images
        a8 = arr_t.astype(E4)
        d8 = (arr_t - a8.astype(np.float32)).astype(E5)
        return img_pair(a8, f), img_pair(d8, f)

    in_maps = []
    for c in range(N_CORES):
        b, g = divmod(c, NHL)
        sl = slice(g * E, (g + 1) * E)
        x8, dx8 = split8(x32[b].T, S)
        wq8, dwq8 = split8(np.ascontiguousarray(Wqkv[sl, :]).T, E)
        wk8, dwk8 = split8(np.ascontiguousarray(Wqkv[D:][sl, :]).T, E)
        wv8, dwv8 = split8(np.ascontiguousarray(Wqkv[2 * D :][sl, :]).T, E)
        in_maps.append(
            {
                "x8_img": x8,
                "dx8_img": dx8,
                "wq8_img": wq8,
                "dwq8_img": dwq8,
                "wk8_img": wk8,
                "dwk8_img": dwk8,
                "wv8_img": wv8,
                "dwv8_img": dwv8,
                "wo_img": img_kpm(np.ascontiguousarray(Wo[:, sl]).T, 2, D),
                "tri": tri,
                "ones4": np.ones((128, NHL), dtype=np.float32),
            }
        )

    res = run_bass_kernel_spmd(_get_nc(), in_maps, core_ids=list(range(N_CORES)))
    outs = [res.results[c]["out"].astype(np.float32) for c in range(N_CORES)]
    return np.stack(
        [outs[0] + outs[1] + outs[2] + outs[3], outs[4] + outs[5] + outs[6] + outs[7]],
        axis=0,
    )


# revision 7
# speedup vs baseline: 1.1898x; 1.0038x over previous
"""Causal MHSA Trainium2 kernel (8 NeuronCores) — v4.

Sharding: core c = 4*b + g handles batch b and head-group g (4 of 16
heads); host sums the 4 head-group partial projections per batch.

v4 (vs v3):
- All streaming tensors are bf16: x / Wq / Wk / Wv / Wo inputs arrive as
  host-prepared bf16 SBUF images (one strided DMA each, 2-4KB rows), and
  the output partial is written bf16 (host upcasts and sums). Total DMA
  drops from ~20MB to ~8MB per core, shrinking the startup window and the
  tail drain.
- q_T/k_T/ctx_T live in bf16, so the diagonal score matmuls no longer
  need >=256-col widening (bf16 runs 1 cycle/col at any width).
- Output staging is always through SBUF (bf16), never direct from PSUM.
"""

import json

import ml_dtypes
import numpy as np

import concourse.bass as bass
import concourse.mybir as mybir
import concourse.tile as tile
from concourse.bass_utils import run_bass_kernel_spmd

# ---------------------------------------------------------------------------
# Workaround: this container's walrus rejects instructions carrying more
# than one semaphore wait ("Too many sync wait commands", e.g. on the
# TileContext final drain). Split every multi-wait instruction into
# single-wait NoOps on the same engine placed immediately before it.
# ---------------------------------------------------------------------------


def _split_multiwait_bir(bir_bytes: bytes) -> bytes:
    bir = json.loads(bir_bytes)
    ctr = 0
    for fn in bir.get("functions", []):
        for bb in fn.get("blocks", []):
            out = []
            for inst in bb.get("instructions", []):
                si = inst.get("sync_info")
                waits = (si or {}).get("on_wait") or []
                if len(waits) > 1 and "engine" in inst:
                    for w in waits:
                        ctr += 1
                        out.append(
                            {
                                "debug": inst.get("debug", 0),
                                "engine": inst["engine"],
                                "ins": [],
                                "outs": [],
                                "name": f"{inst['name']}-sw{ctr}",
                                "opcode": "NoOp",
                                "sync_info": {"on_update": [], "on_wait": [w]},
                            }
                        )
                    si["on_wait"] = []
                out.append(inst)
            bb["instructions"] = out
    return json.dumps(bir).encode()


class _BassSplitWaits(bass.Bass):
    def to_json_bytes(self) -> bytes:
        return _split_multiwait_bir(super().to_json_bytes())


# ---------------------------------------------------------------------------
B = 2
S = 2048
D = 1024
HD = 64
N_CORES = 8
NHL = 4  # heads per core
E = NHL * HD  # 256
DT = D // 128  # 8
ST = S // 128  # 16
QBS = 512
NQB = S // QBS  # 4
F32 = mybir.dt.float32
F32R = mybir.dt.float32r
BF16 = mybir.dt.bfloat16
E4M3 = mybir.dt.float8e4
E5M2 = mybir.dt.float8e5
DR = mybir.MatmulPerfMode.DoubleRow
SCALE = 1.0 / np.sqrt(HD)


def build_nc() -> bass.Bass:
    nc = _BassSplitWaits()

    # host-prepared SBUF images (partition-major). QKV runs as fp8
    # DoubleRow with residual compensation: W^T x ~= W8^T x8 + W8^T dx8 +
    # dW8^T x8, where *8 are e4m3 and d* are e5m2 residuals (r = full - *8).
    # Layouts pair kt tiles for DoubleRow: x images are [p, j, t, s]
    # (kt = 2j + t), w images [p, j, t, e].
    NJ = DT // 2  # 4 kt-pairs
    x8_img = nc.dram_tensor("x8_img", [128, DT * S], E4M3, kind="ExternalInput")
    dx8_img = nc.dram_tensor("dx8_img", [128, DT * S], E5M2, kind="ExternalInput")
    w8_imgs = {
        w: nc.dram_tensor(f"{w}8_img", [128, DT * E], E4M3, kind="ExternalInput")
        for w in ("wq", "wk", "wv")
    }
    dw8_imgs = {
        w: nc.dram_tensor(f"d{w}8_img", [128, DT * E], E5M2, kind="ExternalInput")
        for w in ("wq", "wk", "wv")
    }
    wo_img = nc.dram_tensor("wo_img", [128, 2 * D], BF16, kind="ExternalInput")
    tri_in = nc.dram_tensor("tri", [128, 128], F32R, kind="ExternalInput")
    ones_in = nc.dram_tensor("ones4", [128, NHL], F32R, kind="ExternalInput")
    out = nc.dram_tensor("out", [S, D], BF16, kind="ExternalOutput")

    def dram_ap(t, base, ap):
        ref = t[0:1, 0:1]
        return bass.AP(tensor=ref.tensor, offset=base, ap=[list(a) for a in ap])

    with tile.TileContext(nc) as tc:
        with (
            tc.tile_pool(name="persist", bufs=1) as pp,
            tc.tile_pool(name="work", bufs=3) as wp,
            tc.tile_pool(name="ps", bufs=1, space="PSUM") as ps,
        ):
            # ---- mega tiles ----
            xm8 = pp.tile([128, DT * S], E4M3, name="xm8", tag="xm8")
            xm84 = xm8.rearrange("p (j t s) -> p j t s", j=NJ, t=2)
            dxm8 = pp.tile([128, DT * S], E5M2, name="dxm8", tag="dxm8")
            dxm84 = dxm8.rearrange("p (j t s) -> p j t s", j=NJ, t=2)
            w84 = {}
            dw84 = {}
            for w in ("wq", "wk", "wv"):
                t8 = pp.tile([128, DT * E], E4M3, name=f"{w}8", tag=f"{w}8")
                w84[w] = t8.rearrange("p (j t e) -> p j t e", j=NJ, t=2)
                td = pp.tile([128, DT * E], E5M2, name=f"d{w}8", tag=f"d{w}8")
                dw84[w] = td.rearrange("p (j t e) -> p j t e", j=NJ, t=2)
            wom = pp.tile([128, 2 * D], BF16, name="wom", tag="wom")
            wom3 = wom.rearrange("p (d c) -> p d c", d=2)
            tri = pp.tile([128, 128], F32R, name="tri", tag="tri")
            tri_bf = pp.tile([128, 128], BF16, name="tri_bf", tag="tri_bf")
            ones_col = pp.tile([128, NHL], F32R, name="ones_col", tag="ones_col")

            # ---- loads: strided DMAs straight off the host images, in
            # first-use order. x on Pool/SWDGE, weights on SP/HWDGE,
            # constants on the ACT queue.
            def x_dma(img, dst4, jlo, jhi, slo, shi):
                nc.gpsimd.dma_start(
                    out=dst4[:, jlo:jhi, :, slo:shi],
                    in_=dram_ap(
                        img,
                        jlo * 2 * S + slo,
                        [[DT * S, 128], [S, 2 * (jhi - jlo)], [1, shi - slo]],
                    ),
                )

            def w_dma(wdram, dst4, jlo, jhi):
                nc.sync.dma_start(
                    out=dst4[:, jlo:jhi, :, :],
                    in_=dram_ap(
                        wdram,
                        jlo * 2 * E,
                        [[DT * E, 128], [1, 2 * (jhi - jlo) * E]],
                    ),
                )

            w_dma(w8_imgs["wq"], w84["wq"], 0, 2)
            x_dma(x8_img, xm84, 0, 2, 0, QBS)
            w_dma(w8_imgs["wq"], w84["wq"], 2, 4)
            x_dma(x8_img, xm84, 2, 4, 0, QBS)
            w_dma(w8_imgs["wk"], w84["wk"], 0, 4)
            x_dma(dx8_img, dxm84, 0, 2, 0, QBS)
            w_dma(dw8_imgs["wq"], dw84["wq"], 0, 4)
            x_dma(dx8_img, dxm84, 2, 4, 0, QBS)
            w_dma(dw8_imgs["wk"], dw84["wk"], 0, 4)
            w_dma(w8_imgs["wv"], w84["wv"], 0, 4)
            w_dma(dw8_imgs["wv"], dw84["wv"], 0, 4)
            nc.scalar.dma_start(out=tri, in_=tri_in[:, :])
            nc.scalar.dma_start(out=ones_col, in_=ones_in[:, :])
            nc.vector.tensor_copy(out=tri_bf, in_=tri)
            x_dma(x8_img, xm84, 0, 4, QBS, 2 * QBS)
            x_dma(dx8_img, dxm84, 0, 4, QBS, 2 * QBS)
            nc.sync.dma_start(
                out=wom3[:, :, :],
                in_=dram_ap(wo_img, 0, [[2 * D, 128], [1, 2 * D]]),
            )
            x_dma(x8_img, xm84, 0, 4, 2 * QBS, 3 * QBS)
            x_dma(dx8_img, dxm84, 0, 4, 2 * QBS, 3 * QBS)
            x_dma(x8_img, xm84, 0, 4, 3 * QBS, 4 * QBS)
            x_dma(dx8_img, dxm84, 0, 4, 3 * QBS, 4 * QBS)

            # ---- persistent intermediates ----
            q_T = [pp.tile([128, S], BF16, name=f"qT{p}", tag=f"qT{p}") for p in range(2)]
            k_T = [pp.tile([128, S], BF16, name=f"kT{p}", tag=f"kT{p}") for p in range(2)]
            v_aug = [
                pp.tile([128, NHL * (HD + 1)], BF16, name=f"va{st}", tag=f"va{st}")
                for st in range(ST)
            ]
            ctx_T = [pp.tile([128, S], BF16, name=f"cT{p}", tag=f"cT{p}") for p in range(2)]

            # ---- unit builders: each unit is ~2 DoubleRow matmuls or one
            # copy. Projections accumulate 3 compensated fp8 terms:
            # W8^T x8 + W8^T dx8 + dW8^T x8 (12 DR matmuls over 4 kt-pairs),
            # ordered mains-first so the residual images can arrive later.
            def qk_units(p, nb, wkey, dst):
                sl = slice(nb * QBS, (nb + 1) * QBS)
                w4, dw4 = w84[wkey], dw84[wkey]
                psl = slice(p * 128, (p + 1) * 128)
                cell = {}

                def terms(j):
                    return (
                        (w4[:, j, :, psl], xm84[:, j, :, sl]),
                        (w4[:, j, :, psl], dxm84[:, j, :, sl]),
                        (dw4[:, j, :, psl], xm84[:, j, :, sl]),
                    )

                # (term, j) emission order: mains j0..3, then residuals
                order = [(0, j) for j in range(NJ)]
                order += [(t, j) for j in range(NJ) for t in (1, 2)]

                def mk(lo, hi):
                    def u():
                        if lo == 0:
                            cell["acc"] = ps.tile(
                                [128, QBS], F32, name="acc", tag="qk", bufs=2
                            )
                        for i in range(lo, hi):
                            t, j = order[i]
                            lh, rh = terms(j)[t]
                            nc.tensor.matmul(
                                cell["acc"],
                                lhsT=lh,
                                rhs=rh,
                                start=(i == 0),
                                stop=(i == len(order) - 1),
                                perf_mode=DR,
                            )

                    return u

                units = [mk(0, 2), mk(2, 4), mk(4, 6), mk(6, 8), mk(8, 10), mk(10, 12)]

                def fin():
                    nc.vector.tensor_copy(out=dst[p][:, sl], in_=cell["acc"])

                units.append(fin)
                return units

            def v_units(st):
                stsl = slice(st * 128, (st + 1) * 128)
                cell = {}

                def terms(j):
                    return (
                        (xm84[:, j, :, stsl], w84["wv"][:, j, :, :]),
                        (dxm84[:, j, :, stsl], w84["wv"][:, j, :, :]),
                        (xm84[:, j, :, stsl], dw84["wv"][:, j, :, :]),
                    )

                order = [(0, j) for j in range(NJ)]
                order += [(t, j) for j in range(NJ) for t in (1, 2)]

                def mk(lo, hi):
                    def u():
                        if lo == 0:
                            cell["acc"] = ps.tile(
                                [128, QBS], F32, name="acc", tag="qk", bufs=2
                            )
                        for i in range(lo, hi):
                            t, j = order[i]
                            lh, rh = terms(j)[t]
                            nc.tensor.matmul(
                                cell["acc"][:, 0:E],
                                lhsT=lh,
                                rhs=rh,
                                start=(i == 0),
                                stop=(i == len(order) - 1),
                                perf_mode=DR,
                            )

                    return u

                units = [mk(0, 4), mk(4, 8), mk(8, 12)]

                def fin():
                    va = v_aug[st].rearrange("p (h c) -> p h c", h=NHL)
                    nc.vector.tensor_copy(
                        out=va[:, :, 0:HD],
                        in_=cell["acc"][:, 0:E].rearrange("p (h c) -> p h c", h=NHL),
                    )
                    nc.vector.tensor_copy(
                        out=va[:, :, HD : HD + 1],
                        in_=ones_col.rearrange("p (h c) -> p h c", c=1),
                    )

                units.append(fin)
                return units

            eng_mode = {"drain": False}

            def outproj_units(st, tag="qk", copy_eng=None, tail=False):
                cell = {}

                def mk_mm(nb):
                    def u():
                        pso = ps.tile([128, QBS], F32, name="pso", tag=tag, bufs=2)
                        cell[nb] = pso
                        for dt_ in range(2):
                            nc.tensor.matmul(
                                pso,
                                lhsT=ctx_T[dt_][:, st * 128 : (st + 1) * 128],
                                rhs=wom3[:, dt_, nb * QBS : (nb + 1) * QBS],
                                start=(dt_ == 0),
                                stop=(dt_ == 1),
                            )

                    return u

                def mk_fin(nb, eng):
                    def u():
                        # stage via SBUF bf16 (frees the PSUM slot fast) and
                        # DMA the half right away so the tail's last DMA
                        # chain is short
                        if "osb" not in cell:
                            cell["osb"] = wp.tile(
                                [128, D], BF16, name="osb", tag="osb", bufs=4
                            )
                        if eng == "scalar":
                            nc.scalar.copy(
                                out=cell["osb"][:, nb * QBS : (nb + 1) * QBS],
                                in_=cell[nb],
                            )
                        else:
                            nc.vector.tensor_copy(
                                out=cell["osb"][:, nb * QBS : (nb + 1) * QBS],
                                in_=cell[nb],
                            )
                        # tail: odd-nb DMAs go out the SWDGE (Pool) queue so
                        # the 625ns/DMA HWDGE generation chain halves
                        dma_q = nc.gpsimd if (tail and nb == 1) else nc.sync
                        dma_q.dma_start(
                            out=out[st * 128 : (st + 1) * 128, nb * QBS : (nb + 1) * QBS],
                            in_=cell["osb"][:, nb * QBS : (nb + 1) * QBS],
                        )

                    return u

                if tail:
                    # both matmuls back-to-back (alternating PSUM tags give 4
                    # slots), staging copies split across ACT and DVE, and a
                    # single full-row DMA per st (fewer DGE generations on
                    # the critical tail)
                    def copy_only(nb, eng):
                        def u():
                            if "osb" not in cell:
                                cell["osb"] = wp.tile(
                                    [128, D], BF16, name="osb", tag="osb", bufs=4
                                )
                            dst = cell["osb"][:, nb * QBS : (nb + 1) * QBS]
                            if eng == "scalar":
                                nc.scalar.copy(out=dst, in_=cell[nb])
                            else:
                                nc.vector.tensor_copy(out=dst, in_=cell[nb])

                        return u

                    def full_dma():
                        def u():
                            (nc.gpsimd if st % 2 == 0 else nc.sync).dma_start(
                                out=out[st * 128 : (st + 1) * 128, :],
                                in_=cell["osb"],
                            )

                        return u

                    return [
                        mk_mm(0),
                        mk_mm(1),
                        copy_only(0, "scalar"),
                        copy_only(1, "vector"),
                        full_dma(),
                    ]
                return [mk_mm(0), mk_fin(0, copy_eng), mk_mm(1), mk_fin(1, copy_eng)]

            # ---- attention block with deferred normalization ----
            def attention(p, qb, fillers=(), last=False):
                fillers = list(fillers)
                n_kt = 4 * qb + 4
                ctxs = [
                    ps.tile([128, QBS], F32, name=f"ctx{h}", tag="ctx", bufs=2)
                    for h in range(2)
                ]
                pts = {}
                for kt in range(n_kt + 1):
                    if kt < n_kt:
                        o = 0 if kt < 4 * qb else (kt - 4 * qb) * 128
                        s_ps = ps.tile([128, 2 * QBS], F32, name="s_ps", tag="s", bufs=2)
                        for hl in range(2):
                            nc.tensor.matmul(
                                s_ps[:, hl * QBS + o : (hl + 1) * QBS],
                                lhsT=k_T[p][
                                    hl * HD : (hl + 1) * HD, kt * 128 : (kt + 1) * 128
                                ],
                                rhs=q_T[p][
                                    hl * HD : (hl + 1) * HD,
                                    qb * QBS + o : (qb + 1) * QBS,
                                ],
                                start=True,
                                stop=True,
                            )
                        pt = wp.tile([128, 2 * QBS], BF16, name="pt", tag="pt", bufs=4)
                        sv = s_ps.rearrange("p (h q) -> p h q", h=2)
                        pv = pt.rearrange("p (h q) -> p h q", h=2)
                        nc.scalar.activation(
                            out=pv[:, :, o:QBS],
                            in_=sv[:, :, o:QBS],
                            func=mybir.ActivationFunctionType.Exp,
                            scale=float(SCALE),
                        )
                        if kt >= 4 * qb:
                            for hl in range(2):
                                blk = pt[:, hl * QBS + o : hl * QBS + o + 128]
                                nc.vector.tensor_mul(blk, blk, tri_bf)
                        pts[kt] = (pt, o)
                    # fillers BEFORE ctx(kt-1): PE executes in order, so the
                    # (independent) fillers run while exp(kt-1) finishes; the
                    # ctx matmul then starts without exposing the ACT latency.
                    # Pops adapt so the filler list drains evenly across the
                    # block instead of leaving a burst stuck behind the last
                    # (dependency-carrying) ctx matmul.
                    iters_left = n_kt + 1 - kt
                    if last:
                        npop = 1
                    else:
                        npop = max(2, -(-len(fillers) // iters_left))
                    for _ in range(npop):
                        if fillers:
                            fillers.pop(0)()
                    if kt > 0:
                        pt, o = pts.pop(kt - 1)
                        for hl in range(2):
                            nc.tensor.matmul(
                                ctxs[hl][0 : HD + 1, o:QBS],
                                lhsT=v_aug[kt - 1][
                                    :, (2 * p + hl) * (HD + 1) : (2 * p + hl + 1) * (HD + 1)
                                ],
                                rhs=pt[:, hl * QBS + o : (hl + 1) * QBS],
                                start=(kt - 1 == 0),
                                stop=(kt - 1 == n_kt - 1),
                                skip_group_check=True,
                            )
                # stage unnormalized ctx through SBUF + reciprocal on the
                # denominator row; the broadcast+multiply is deferred
                cuns = []
                for hl in range(2):
                    cun = wp.tile([HD + 1, QBS], F32R, name="cun", tag="cun", bufs=4)
                    if last:
                        # keep the tail's DVE budget for recips/norm muls and
                        # staging copies; ACT has no exps left here
                        nc.scalar.copy(out=cun, in_=ctxs[hl][0 : HD + 1, :])
                    else:
                        nc.vector.tensor_copy(out=cun, in_=ctxs[hl][0 : HD + 1, :])
                    # in-place reciprocal at partition 64 (equal in/out base —
                    # a DVE input at partition 64 with output at partition 0
                    # reads wrong data on HW)
                    with nc.allow_low_precision(reason="f32r is bitwise f32"):
                        nc.vector.reciprocal(
                            out=cun[HD : HD + 1, :], in_=cun[HD : HD + 1, :]
                        )
                    cuns.append(cun)
                if last:
                    # post-loop drain copies go to ACT so DVE stays clear
                    # for the norm chain
                    eng_mode["drain"] = True
                while fillers:
                    fillers.pop(0)()

                bcs = {}

                def mk_bc(hl):
                    cun = cuns[hl]

                    def u():
                        # broadcast recip row across 64 partitions with a
                        # 1-row matmul: ones(1,64)^T @ recip(1,QBS)
                        bc = ps.tile(
                            [128, QBS], F32, name="bc",
                            tag=("s" if last else "qk"), bufs=2,
                        )
                        bcs[hl] = bc
                        # tri row 64 cols 64:128 is all-ones at partition 64,
                        # matching the recip row's base partition
                        nc.tensor.matmul(
                            bc[0:HD, :],
                            lhsT=tri[HD : HD + 1, HD : 2 * HD],
                            rhs=cun[HD : HD + 1, :],
                            start=True,
                            stop=True,
                        )

                    return u

                def mk_mul(hl, sl_):
                    cun = cuns[hl]

                    def u():
                        nc.vector.tensor_mul(
                            ctx_T[p][
                                hl * HD : (hl + 1) * HD,
                                qb * QBS + sl_.start : qb * QBS + sl_.stop,
                            ],
                            cun[0:HD, sl_],
                            bcs[hl][0:HD, sl_],
                        )

                    return u

                def mk_norm(hl):
                    bcu = mk_bc(hl)
                    mulu = mk_mul(hl, slice(0, QBS))

                    def u():
                        bcu()
                        mulu()

                    return u

                if last:
                    # column-split muls: the tail's st12/13 need only the
                    # first half of the qb3 columns
                    return [
                        mk_bc(0),
                        mk_bc(1),
                        mk_mul(0, slice(0, 256)),
                        mk_mul(1, slice(0, 256)),
                        mk_mul(0, slice(256, QBS)),
                        mk_mul(1, slice(256, QBS)),
                    ]
                return [mk_norm(0), mk_norm(1)]

            def with_norm(units, norm):
                units = list(units)
                return units[:4] + list(norm) + units[4:]

            # ---- emission schedule ----
            # startup: mains (w8+x8 only) before residuals so PE consumes in
            # DMA-arrival order; two accs alive at a time (qk tag bufs=2)
            qg = qk_units(0, 0, "wq", q_T)
            kg = qk_units(0, 0, "wk", k_T)
            for u in (qg[0], qg[1], kg[0], kg[1]):
                u()
            for u in qg[2:]:
                u()
            for u in kg[2:]:
                u()
            vg = [v_units(st) for st in range(4)]
            vg[0][0]()
            vg[1][0]()
            for u in vg[0][1:]:
                u()
            for u in vg[1][1:]:
                u()
            vg[2][0]()
            vg[3][0]()
            for u in vg[2][1:]:
                u()
            for u in vg[3][1:]:
                u()
            # att(0,0) gets fillers so its ACT-serial warmup doesn't stall PE;
            # qk(0,1) drains late enough that x block 1 has landed
            n00 = attention(
                0, 0,
                qk_units(1, 0, "wq", q_T) + qk_units(1, 0, "wk", k_T)
                + qk_units(0, 1, "wq", q_T),
            )
            # v(4..7) must be scheduled a block BEFORE att(0,1) reads them
            n10 = attention(
                1, 0,
                with_norm(
                    qk_units(0, 1, "wk", k_T)
                    + v_units(4) + v_units(5) + v_units(6) + v_units(7),
                    n00,
                ),
            )
            n01 = attention(
                0, 1,
                with_norm(
                    qk_units(1, 1, "wq", q_T) + qk_units(1, 1, "wk", k_T), n10
                ),
            )
            n11 = attention(
                1, 1,
                with_norm(
                    qk_units(0, 2, "wq", q_T) + qk_units(0, 2, "wk", k_T)
                    + v_units(8) + v_units(9) + v_units(10) + v_units(11),
                    n01,
                ),
            )
            n02 = attention(
                0, 2,
                with_norm(
                    qk_units(1, 2, "wq", q_T) + qk_units(1, 2, "wk", k_T), n11
                ),
            )
            n12 = attention(
                1, 2,
                with_norm(
                    qk_units(0, 3, "wq", q_T) + qk_units(0, 3, "wk", k_T)
                    + v_units(12) + v_units(13) + v_units(14) + v_units(15)
                    + outproj_units(0) + outproj_units(1),
                    n02,
                ),
            )
            n03 = attention(
                0, 3,
                with_norm(
                    qk_units(1, 3, "wq", q_T) + qk_units(1, 3, "wk", k_T)
                    + outproj_units(2) + outproj_units(3)
                    + outproj_units(4) + outproj_units(5),
                    n12,
                ),
            )
            n13 = attention(
                1, 3,
                with_norm(
                    outproj_units(6) + outproj_units(7) + outproj_units(8)
                    + outproj_units(9) + outproj_units(10) + outproj_units(11),
                    n03,
                ),
                last=True,
            )

            for u in n13[0:4]:
                u()
            # tail: alternate PSUM tags (ctx tag is free now) for a 4-slot
            # rotation; copies split across ACT/DVE, DMAs across HWDGE/SWDGE;
            # st12/13 go right after the first-half norm muls
            for st in (12, 13):
                for u in outproj_units(
                    st, tag=("qk" if st % 2 == 0 else "ctx"), tail=True
                ):
                    u()
            n13[4]()
            n13[5]()
            for st in (14, 15):
                for u in outproj_units(
                    st, tag=("qk" if st % 2 == 0 else "ctx"), tail=True
                ):
                    u()
    return nc


_NC_CACHE = {}


def _get_nc() -> bass.Bass:
    if "nc" not in _NC_CACHE:
        _NC_CACHE["nc"] = build_nc()
    return _NC_CACHE["nc"]


def kernel(in_features: np.ndarray, Wqkv: np.ndarray, Wo: np.ndarray) -> np.ndarray:
    BF = ml_dtypes.bfloat16
    E4 = ml_dtypes.float8_e4m3
    E5 = ml_dtypes.float8_e5m2
    NJ = DT // 2
    x32 = np.ascontiguousarray(np.asarray(in_features, dtype=np.float32))
    Wqkv = np.asarray(Wqkv, dtype=np.float32)
    Wo = np.asarray(Wo, dtype=np.float32)

    tri = np.triu(np.ones((128, 128), dtype=np.float32))  # P^T[k,q] valid iff q >= k

    def img_kpm(arr_t, k, f):
        # arr_t: [k*128, f] -> partition-major image [128, k*f]
        return np.ascontiguousarray(
            arr_t.reshape(k, 128, f).transpose(1, 0, 2).reshape(128, k * f).astype(BF)
        )

    def img_pair(a, f):
        # [DT*128, f] -> kt-paired partition-major image [128, NJ*2*f]
        return np.ascontiguousarray(
            a.reshape(NJ, 2, 128, f).transpose(2, 0, 1, 3).reshape(128, DT * f)
        )

    def split8(arr_t, f):
        # fp8 main (e4m3) + residual (e5m2) pair-layout images
        a8 = arr_t.astype(E4)
        d8 = (arr_t - a8.astype(np.float32)).astype(E5)
        return img_pair(a8, f), img_pair(d8, f)

    in_maps = []
    for c in range(N_CORES):
        b, g = divmod(c, NHL)
        sl = slice(g * E, (g + 1) * E)
        x8, dx8 = split8(x32[b].T, S)
        wq8, dwq8 = split8(np.ascontiguousarray(Wqkv[sl, :]).T, E)
        wk8, dwk8 = split8(np.ascontiguousarray(Wqkv[D:][sl, :]).T, E)
        wv8, dwv8 = split8(np.ascontiguousarray(Wqkv[2 * D :][sl, :]).T, E)
        in_maps.append(
            {
                "x8_img": x8,
                "dx8_img": dx8,
                "wq8_img": wq8,
                "dwq8_img": dwq8,
                "wk8_img": wk8,
                "dwk8_img": dwk8,
                "wv8_img": wv8,
                "dwv8_img": dwv8,
                "wo_img": img_kpm(np.ascontiguousarray(Wo[:, sl]).T, 2, D),
                "tri": tri,
                "ones4": np.ones((128, NHL), dtype=np.float32),
            }
        )

    res = run_bass_kernel_spmd(_get_nc(), in_maps, core_ids=list(range(N_CORES)))
    outs = [res.results[c]["out"].astype(np.float32) for c in range(N_CORES)]
    return np.stack(
        [outs[0] + outs[1] + outs[2] + outs[3], outs[4] + outs[5] + outs[6] + outs[7]],
        axis=0,
    )
